# revision 1
# baseline (speedup 1.0000x reference)
"""Trainium2 Bass kernel for causal self-attention with RoPE.

Shapes: x (2, 2048, 2048), 16 heads x 128 head_dim.
Sharding: 8 cores = 2 batch x 4 head-groups (4 heads per core).
Each core computes q/k/v projections for its heads, RoPE, causal-masked
softmax attention, and a partial output projection (its head columns of
wo); the host sums the 4 partials per batch element.

Layout strategy (per core):
  - q,k built in transposed layout (head_dim on partitions, t free) so
    RoPE and the score matmuls need no on-device transposes.  The host
    permutes wq/wk columns so RoPE's even/odd pairs become the two
    partition halves, and pre-scales wq by 1/sqrt(head_dim).
  - scores computed as s^T (keys x q) per 256-query group; softmax skips
    the max-subtraction (scores are O(1) by construction); row sums via
    an ones-vector matmul; normalization folded into the PSUM eviction.
  - v computed directly in (t x e) layout by using x^T as the stationary
    operand, so the p@v matmul needs no transposes anywhere.
  - fully-masked key blocks are skipped (host inspects the mask), which
    halves the attention work for the causal mask; deduplicated mask
    tiles are added only where a block is partially masked.
  - matmuls run in float32r (tf32) which streams at full rate for moving
    dims >= 256.
  - weights/mask stream on the scalar-engine DMA queue, x^T/tables/output
    on the sync-engine queue, so activations never queue behind weights.
"""

import sys
from contextlib import ExitStack

if "/opt/trn_rl_repo" not in sys.path:
    sys.path.insert(0, "/opt/trn_rl_repo")

import numpy as np

import concourse.bacc as bacc
import concourse.mybir as mybir
import concourse.tile as tile
from concourse.bass_utils import run_bass_kernel_spmd

B, T, D, NH, HD = 2, 2048, 2048, 16, 128
HPC = 4              # heads per core
PAIR = 256           # queries per group
NPAIR = T // PAIR    # 8
NCHUNK = T // HD     # 16 key chunks of 128
NSLICE = T // PAIR   # 8 t-slices for projections
F32R = mybir.dt.float32r
F32 = mybir.dt.float32
MASK_PRELOAD_MAX = 24


def _mask_structure(mask):
    """Classify each (query-group, key-chunk) block of the additive mask.

    Returns (statuses, maskt): statuses[j] is a list of
    (chunk, mask_tile_index_or_minus1) for blocks that must be computed;
    maskt is the packed (128, nmask, 256) array of deduplicated
    transposed mask tiles for partially-masked blocks.
    """
    statuses = []
    tiles = {}
    tile_list = []
    for j in range(NPAIR):
        q = slice(j * PAIR, (j + 1) * PAIR)
        lst = []
        for c in range(NCHUNK):
            k = slice(c * HD, (c + 1) * HD)
            sub = mask[q, k]
            if np.all(sub <= -1e8):
                continue
            if np.all(sub == 0.0):
                lst.append((c, -1))
            else:
                key = sub.tobytes()
                mi = tiles.get(key)
                if mi is None:
                    mi = len(tile_list)
                    tiles[key] = mi
                    tile_list.append(np.ascontiguousarray(sub.T))
                lst.append((c, mi))
        assert lst, f"query group {j} has every key block masked"
        statuses.append(lst)
    nmask = max(1, len(tile_list))
    maskt = np.zeros((HD, nmask, PAIR), np.float32)
    for i, t in enumerate(tile_list):
        assert np.all(t <= 64.0), "additive mask too large for exp-mask trick"
        maskt[:, i, :] = np.exp(t)
    return statuses, maskt


def _build_program(statuses, nmask):
    nc = bacc.Bacc(None, target_bir_lowering=False)

    xt_d = nc.dram_tensor("xt", [D, T], F32R, kind="ExternalInput")
    wq_d = nc.dram_tensor("wqt", [D, HPC * HD], F32R, kind="ExternalInput")
    wk_d = nc.dram_tensor("wkt", [D, HPC * HD], F32R, kind="ExternalInput")
    wv_d = nc.dram_tensor("wvt", [D, HPC * HD], F32R, kind="ExternalInput")
    wo_d = nc.dram_tensor("wot", [HPC * HD, D], F32R, kind="ExternalInput")
    cs_d = nc.dram_tensor("cs", [HD, 2, T], F32, kind="ExternalInput")
    mk_d = nc.dram_tensor("maskt", [HD, nmask, PAIR], F32, kind="ExternalInput")
    ones_d = nc.dram_tensor("ones_col", [HD, 1], F32R, kind="ExternalInput")
    onesr_d = nc.dram_tensor("ones_row", [1, HD], F32R, kind="ExternalInput")
    out_d = nc.dram_tensor("out", [T, D], F32, kind="ExternalOutput")

    xt_ap = xt_d.ap().rearrange("(k p) t -> p k t", p=HD)
    wq_ap = wq_d.ap().rearrange("(k p) e -> p k e", p=HD)
    wk_ap = wk_d.ap().rearrange("(k p) e -> p k e", p=HD)
    wv_ap = wv_d.ap().rearrange("(k p) e -> p k e", p=HD)
    wo_ap = wo_d.ap().rearrange("(h p) e -> p h e", p=HD)
    EXP = mybir.ActivationFunctionType.Exp
    preload_mask = nmask <= MASK_PRELOAD_MAX

    with tile.TileContext(nc) as tc, ExitStack() as top:
        constp = top.enter_context(tc.tile_pool(name="const", bufs=1))
        ones_sb = constp.tile([HD, 1], F32R)
        onesr_sb = constp.tile([1, HD], F32R)
        nc.scalar.dma_start(ones_sb[:], ones_d[:])
        nc.scalar.dma_start(onesr_sb[:], onesr_d[:])

        qkp = top.enter_context(tc.tile_pool(name="qkp", bufs=1))
        # q heads at [:, h, :], k heads at [:, 4+h, :]
        qk_sb = qkp.tile([HD, 2 * HPC, T], F32R)

        # wv + xt pools span the q/k pass (prefetch) and the v pass
        with ExitStack() as vph:
            wvp = vph.enter_context(tc.tile_pool(name="wvp", side="right", bufs=1))
            wv_sb = wvp.tile([HD, NCHUNK, HPC * HD], F32R)
            xtp = vph.enter_context(tc.tile_pool(name="xtp", side="right", bufs=2))

            # ---- combined q/k projection pass (+ fused RoPE) ----
            with ExitStack() as ph:
                wp = ph.enter_context(tc.tile_pool(name="wp", side="right", bufs=1))
                csp = ph.enter_context(tc.tile_pool(name="csp", side="right", bufs=2))
                ropep = ph.enter_context(tc.tile_pool(name="ropep", side="right", bufs=2))
                pps = ph.enter_context(tc.tile_pool(name="pps", bufs=6, space="PSUM"))
                wqk_sb = wp.tile([HD, 2, NCHUNK, HPC * HD], F32R)
                # weights on the scalar queue, split per k-chunk so the
                # first matmuls start as soon as chunk 0 lands
                for k in range(NCHUNK):
                    nc.scalar.dma_start(wqk_sb[:, 0, k, :], wq_ap[:, k, :])
                for k in range(NCHUNK):
                    nc.scalar.dma_start(wqk_sb[:, 1, k, :], wk_ap[:, k, :])
                for k in range(NCHUNK):  # prefetch wv for the next pass
                    nc.scalar.dma_start(wv_sb[:, k, :], wv_ap[:, k, :])
                for ns in range(NSLICE):
                    tsl = slice(ns * PAIR, (ns + 1) * PAIR)
                    xt = xtp.tile([HD, NCHUNK, PAIR], F32R, tag="xt")
                    nc.sync.dma_start(xt[:], xt_ap[:, :, tsl])
                    cs_sl = csp.tile([HD, 2, PAIR], F32, tag="cs")
                    nc.sync.dma_start(cs_sl[:], cs_d[:, :, tsl])
                    for wsel in range(2):
                        for h in range(HPC):
                            ps = pps.tile([HD, PAIR], F32, tag="ps")
                            hs = slice(h * HD, (h + 1) * HD)
                            for k in range(NCHUNK):
                                nc.tensor.matmul(
                                    ps[:],
                                    wqk_sb[:, wsel, k, hs],
                                    xt[:, k, :],
                                    start=(k == 0),
                                    stop=(k == NCHUNK - 1),
                                )
                            # RoPE: dst = raw*C + swap(raw)*S.  The swap is
                            # materialized by two ScalarE half-copies, the S
                            # product runs on GpSimd, so VectorE only does
                            # one multiply and one add per tile.
                            dst = qk_sb[:, wsel * HPC + h, tsl]
                            sw = ropep.tile([HD, PAIR], F32, tag="sw")
                            nc.scalar.copy(sw[0:64, :], ps[64:128, :])
                            nc.scalar.copy(sw[64:128, :], ps[0:64, :])
                            tb = ropep.tile([HD, PAIR], F32R, tag="tb")
                            nc.vector.tensor_mul(dst, ps[:], cs_sl[:, 0, :])
                            nc.vector.tensor_mul(tb[:], sw[:], cs_sl[:, 1, :])
                            nc.vector.tensor_add(dst, dst, tb[:])

            # ---- v projection (normal layout, x^T stationary) ----
            vap = top.enter_context(tc.tile_pool(name="vap", bufs=1))
            v_all = vap.tile([HD, NCHUNK, HPC * HD], F32R)
            with ExitStack() as ph:
                vps = ph.enter_context(tc.tile_pool(name="vps", bufs=4, space="PSUM"))
                for ns in reversed(range(NSLICE)):
                    tsl = slice(ns * PAIR, (ns + 1) * PAIR)
                    xt = xtp.tile([HD, NCHUNK, PAIR], F32R, tag="xt")
                    nc.sync.dma_start(xt[:], xt_ap[:, :, tsl])
                    for tc2 in range(2):
                        ps = vps.tile([HD, HPC * HD], F32, tag="vps")
                        for k in range(NCHUNK):
                            nc.tensor.matmul(
                                ps[:],
                                xt[:, k, tc2 * HD:(tc2 + 1) * HD],
                                wv_sb[:, k, :],
                                start=(k == 0),
                                stop=(k == NCHUNK - 1),
                            )
                        nc.scalar.copy(v_all[:, ns * 2 + tc2, :], ps[:])

        # ---- attention + interleaved output projection ----
        ctxp = top.enter_context(tc.tile_pool(name="ctxp", bufs=1))
        ctx_sb = ctxp.tile([HD, HPC, T], F32R)
        wop = top.enter_context(tc.tile_pool(name="wop", bufs=1))
        wo_sb = wop.tile([HD, HPC, D], F32R)
        with ExitStack() as ph:
            ptp = ph.enter_context(tc.tile_pool(name="ptp", side="right", bufs=2))
            mkp = ph.enter_context(tc.tile_pool(name="mkp", side="right", bufs=4))
            lrp = ph.enter_context(tc.tile_pool(name="lrp", side="right", bufs=2))
            rbp = ph.enter_context(tc.tile_pool(name="rbp", side="right", bufs=2))
            sps = ph.enter_context(tc.tile_pool(name="sps", bufs=2, space="PSUM"))
            ops = ph.enter_context(tc.tile_pool(name="ops", bufs=2, space="PSUM"))
            lps = ph.enter_context(tc.tile_pool(name="lps", bufs=2, space="PSUM"))

            mk_sb = None
            if preload_mask:
                mkpre = ph.enter_context(
                    tc.tile_pool(name="mkpre", side="right", bufs=1)
                )
                mk_sb = mkpre.tile([HD, nmask, PAIR], F32)
                nc.scalar.dma_start(mk_sb[:], mk_d[:])
            for h in range(HPC):  # prefetch wo
                nc.scalar.dma_start(wo_sb[:, h, :], wo_ap[:, h, :])

            def mask_tile(mi):
                if preload_mask:
                    return mk_sb[:, mi, :]
                mt = mkp.tile([HD, PAIR], F32, tag="mk")
                nc.scalar.dma_start(mt[:], mk_d[:, mi, :])
                return mt[:]

            def finalize(st):
                # off the tensor engine: DVE fast-recip -> GpSimd partition
                # broadcast -> DVE multiply into ctx
                lr = lrp.tile([1, PAIR], F32, tag="lr")
                nc.vector.reciprocal_approx_fast(lr[:], st["l"])
                rb_sb = rbp.tile([HD, PAIR], F32, tag="rb")
                nc.gpsimd.partition_broadcast(rb_sb[:], lr[:])
                nc.vector.tensor_mul(
                    ctx_sb[:, st["h"], st["qsl"]], st["o"], rb_sb[:]
                )

            def emit_ol(dq):
                # deferred p@v and row-sum matmuls for an exp'd quad
                pi, quad, st = dq
                h = st["h"]
                for t, (c, mi) in enumerate(quad):
                    nc.tensor.matmul(
                        st["o"],
                        v_all[:, c, h * HD:(h + 1) * HD],
                        st["pt"][:, pi + t, :],
                        start=(st["oi"] == 0),
                        stop=(st["oi"] == st["n"] - 1),
                        skip_group_check=True,
                    )
                    st["oi"] += 1
                for t, (c, mi) in enumerate(quad):
                    nc.tensor.matmul(
                        st["l"],
                        ones_sb[:],
                        st["pt"][:, pi + t, :],
                        start=(st["li"] == 0),
                        stop=(st["li"] == st["n"] - 1),
                        skip_group_check=True,
                    )
                    st["li"] += 1
                return st["li"] == st["n"]

            pending_ol = None
            pending_fin = None
            for j in reversed(range(NPAIR)):
                qsl = slice(j * PAIR, (j + 1) * PAIR)
                chunks = list(reversed(statuses[j]))
                n = len(chunks)
                quads = [chunks[ii:ii + 4] for ii in range(0, n, 4)]
                for h in range(HPC):
                    o_ps = ops.tile([HD, PAIR], F32, tag="o")
                    l_ps = lps.tile([1, PAIR], F32, tag="l")
                    pt = ptp.tile([HD, NCHUNK, PAIR], F32R, tag="pt")
                    st = {"o": o_ps[:], "l": l_ps[:],
                          "pt": pt, "h": h, "qsl": qsl, "n": n,
                          "oi": 0, "li": 0}
                    for qi, quad in enumerate(quads):
                        w = len(quad)
                        s_ps = sps.tile([HD, 4, PAIR], F32, tag="s")
                        for t, (c, mi) in enumerate(quad):
                            nc.tensor.matmul(
                                s_ps[:, t, :],
                                qk_sb[:, HPC + h, c * HD:(c + 1) * HD],
                                qk_sb[:, h, qsl],
                                start=True,
                                stop=True,
                            )
                        nc.scalar.activation(
                            pt[:, qi * 4:qi * 4 + w, :], s_ps[:, 0:w, :], EXP
                        )
                        # multiplicative exp-mask applied to pt
                        # (exp(s+m) == exp(s)*exp(m)), off the exp chain
                        t = 0
                        while t < w:
                            c, mi = quad[t]
                            if mi < 0:
                                t += 1
                                continue
                            r = t + 1
                            while (preload_mask and r < w and quad[r][1] >= 0
                                   and quad[r][1] == quad[r - 1][1] + 1):
                                r += 1
                            if preload_mask:
                                sl = slice(qi * 4 + t, qi * 4 + r)
                                nc.vector.tensor_mul(
                                    pt[:, sl, :], pt[:, sl, :],
                                    mk_sb[:, mi:mi + (r - t), :],
                                )
                            else:
                                sl = slice(qi * 4 + t, qi * 4 + t + 1)
                                nc.vector.tensor_mul(
                                    pt[:, sl, :], pt[:, sl, :], mask_tile(mi)
                                )
                                r = t + 1
                            t = r
                        if pending_ol is not None:
                            if emit_ol(pending_ol):
                                pending_fin = pending_ol[2]
                            pending_ol = None
                        if pending_fin is not None and pending_fin is not st:
                            finalize(pending_fin)
                            pending_fin = None
                        pending_ol = (qi * 4, quad, st)
            if pending_ol is not None:
                if emit_ol(pending_ol):
                    pending_fin = pending_ol[2]
            if pending_fin is not None:
                finalize(pending_fin)
        # ---- output projection ----
        with ExitStack() as ph:
            evp = ph.enter_context(tc.tile_pool(name="evp", side="right", bufs=6))
            wops = ph.enter_context(tc.tile_pool(name="wops", bufs=6, space="PSUM"))
            for tck in range(NCHUNK):
                tsl = slice(tck * HD, (tck + 1) * HD)
                for es in range(4):
                    esl = slice(es * 512, (es + 1) * 512)
                    ps = wops.tile([HD, 512], F32, tag="wo")
                    for h in range(HPC):
                        nc.tensor.matmul(
                            ps[:],
                            ctx_sb[:, h, tsl],
                            wo_sb[:, h, esl],
                            start=(h == 0),
                            stop=(h == HPC - 1),
                        )
                    ev = evp.tile([HD, 512], F32, tag="ev")
                    nc.scalar.copy(ev[:], ps[:])
                    nc.sync.dma_start(out_d[tsl, esl], ev[:])
    nc.compile()
    return nc


_PERM = np.concatenate(
    [np.concatenate([np.arange(0, HD, 2), np.arange(1, HD, 2)]) + h * HD
     for h in range(HPC)]
)


def prepare(x, freqs, mask, wq, wk, wv, wo):
    """Host-side sharding/prep. Returns (nc, in_maps)."""
    x = np.asarray(x, np.float32)
    freqs = np.asarray(freqs, np.float32)
    mask = np.asarray(mask, np.float32)
    wq, wk, wv, wo = (np.asarray(w, np.float32) for w in (wq, wk, wv, wo))

    statuses, maskt = _mask_structure(mask)
    nc = _build_program(statuses, maskt.shape[1])

    scale = np.float32(1.0 / np.sqrt(HD))
    cos = np.ascontiguousarray(freqs[:, :, 0].T)  # (64, T)
    sin = np.ascontiguousarray(freqs[:, :, 1].T)
    cs = np.empty((HD, 2, T), np.float32)
    cs[0:64, 0, :] = cos
    cs[64:128, 0, :] = cos
    cs[0:64, 1, :] = -sin
    cs[64:128, 1, :] = sin

    ones_col = np.ones((HD, 1), np.float32)
    ones_row = np.ones((1, HD), np.float32)
    xt = [np.ascontiguousarray(x[b].T) for b in range(B)]

    in_maps = []
    for core in range(8):
        b, g = core // 4, core % 4
        cols = slice(g * HPC * HD, (g + 1) * HPC * HD)
        in_maps.append({
            "xt": xt[b],
            "wqt": np.ascontiguousarray((wq.T[:, cols] * scale)[:, _PERM]),
            "wkt": np.ascontiguousarray(wk.T[:, cols][:, _PERM]),
            "wvt": np.ascontiguousarray(wv.T[:, cols]),
            "wot": np.ascontiguousarray(wo.T[cols, :]),
            "cs": cs,
            "maskt": maskt,
            "ones_col": ones_col,
            "ones_row": ones_row,
        })
    return nc, in_maps


def run(x, freqs, mask, wq, wk, wv, wo, **spmd_kwargs):
    nc, in_maps = prepare(x, freqs, mask, wq, wk, wv, wo)
    res = run_bass_kernel_spmd(nc, in_maps, list(range(8)), **spmd_kwargs)
    parts = [res.results[c]["out"] for c in range(8)]
    out = np.stack([
        parts[b * 4] + parts[b * 4 + 1] + parts[b * 4 + 2] + parts[b * 4 + 3]
        for b in range(B)
    ]).astype(np.float32)
    return out, res


def kernel(x, freqs, mask, wq, wk, wv, wo):
    out, _ = run(x, freqs, mask, wq, wk, wv, wo)
    return out



# revision 5
# speedup vs baseline: 1.1297x; 1.1297x over previous
"""Trainium2 Bass kernel for causal self-attention with RoPE.

Shapes: x (2, 2048, 2048), 16 heads x 128 head_dim.
Sharding: 8 cores = 2 batch x 4 head-groups (4 heads per core).
Each core computes q/k/v projections for its heads, RoPE, causal-masked
softmax attention, and a partial output projection (its head columns of
wo); the host sums the 4 partials per batch element.

All matmul operands are bf16 (PSUM accumulation stays fp32): bf16 and
fp32r stream at the same 1 cycle/row on the PE, but bf16 halves DMA
traffic and SBUF footprint, loads stationary weights at 1 cycle/row,
and unlocks the DVE 2x packed mode for the element-wise work.

Layout strategy (per core):
  - q,k built in transposed layout (head_dim on partitions, t free) so
    RoPE and the score matmuls need no on-device transposes.  The host
    permutes wq/wk columns so RoPE's even/odd pairs become the two
    partition halves, and pre-scales wq by 1/sqrt(head_dim).
  - projections run in 512-query slices into a 4-bank PSUM slab (one
    bank per head); eviction+pair-swap amortize into three big scalar
    copies per slab, then RoPE is two DVE multiplies and an add per
    head at the 2x bf16 rate.
  - x^T stays resident in SBUF for the whole kernel (bf16 makes it
    fit), so the v pass reuses it without a second HBM read.
  - scores computed as s^T (keys x q) per 256-query group; softmax
    skips the max-subtraction (scores are O(1) by construction); row
    sums via a ones-vector matmul (the PE is the cheapest engine for a
    partition-axis reduction); normalization folded into the PSUM
    eviction on the DVE.
  - fully-masked key blocks are skipped (host inspects the mask);
    deduplicated exp(mask) tiles multiply pt only where a block is
    partially masked.
  - the output projection is interleaved into the attention loop (one
    PSUM group popped after each head finalize), so its matmuls fill
    the PE during softmax dependency stalls instead of forming a
    serial tail, and the tensor engine never idles long enough to
    lose its p-state.
"""

import sys
from contextlib import ExitStack

if "/opt/trn_rl_repo" not in sys.path:
    sys.path.insert(0, "/opt/trn_rl_repo")

import numpy as np

import concourse.bacc as bacc
import concourse.mybir as mybir
import concourse.tile as tile
from concourse.bass_utils import run_bass_kernel_spmd

B, T, D, NH, HD = 2, 2048, 2048, 16, 128
HPC = 4              # heads per core
SL = 512             # projection slice width (max moving dim)
NSL = T // SL        # 4
PAIR = 256           # queries per attention group
NPAIR = T // PAIR    # 8
NCHUNK = T // HD     # 16 key chunks of 128
BF = mybir.dt.bfloat16
F32 = mybir.dt.float32


def _mask_structure(mask):
    """Classify each (query-group, key-chunk) block of the additive mask.

    Returns (statuses, maskt): statuses[j] is a list of
    (chunk, mask_tile_index_or_minus1) for blocks that must be computed;
    maskt is the packed (128, nmask, 256) array of deduplicated
    transposed exp(mask) tiles for partially-masked blocks.
    """
    statuses = []
    tiles = {}
    tile_list = []
    for j in range(NPAIR):
        q = slice(j * PAIR, (j + 1) * PAIR)
        lst = []
        for c in range(NCHUNK):
            k = slice(c * HD, (c + 1) * HD)
            sub = mask[q, k]
            if np.all(sub <= -1e8):
                continue
            if np.all(sub == 0.0):
                lst.append((c, -1))
            else:
                key = sub.tobytes()
                mi = tiles.get(key)
                if mi is None:
                    mi = len(tile_list)
                    tiles[key] = mi
                    tile_list.append(np.ascontiguousarray(sub.T))
                lst.append((c, mi))
        assert lst, f"query group {j} has every key block masked"
        statuses.append(lst)
    nmask = max(1, len(tile_list))
    assert nmask <= 24, "too many distinct mask tiles to preload"
    maskt = np.zeros((HD, nmask, PAIR), np.float32)
    for i, t in enumerate(tile_list):
        assert np.all(t <= 64.0), "additive mask too large for exp-mask trick"
        maskt[:, i, :] = np.exp(t)
    return statuses, maskt


def _build_program(statuses, nmask):
    nc = bacc.Bacc(None, target_bir_lowering=False)

    xt_d = nc.dram_tensor("xt", [D, T], BF, kind="ExternalInput")
    wq_d = nc.dram_tensor("wqt", [D, HPC * HD], BF, kind="ExternalInput")
    wk_d = nc.dram_tensor("wkt", [D, HPC * HD], BF, kind="ExternalInput")
    wv_d = nc.dram_tensor("wvt", [D, HPC * HD], BF, kind="ExternalInput")
    wo_d = nc.dram_tensor("wot", [HPC * HD, D], BF, kind="ExternalInput")
    cs_d = nc.dram_tensor("cs", [HD, 2, T], BF, kind="ExternalInput")
    mk_d = nc.dram_tensor("maskt", [HD, nmask, PAIR], BF, kind="ExternalInput")
    ones_d = nc.dram_tensor("ones_col", [HD, 1], BF, kind="ExternalInput")
    out_d = nc.dram_tensor("out", [T, D], BF, kind="ExternalOutput")

    xt_ap = xt_d.ap().rearrange("(k p) t -> p k t", p=HD)
    wq_ap = wq_d.ap().rearrange("(k p) e -> p k e", p=HD)
    wk_ap = wk_d.ap().rearrange("(k p) e -> p k e", p=HD)
    wv_ap = wv_d.ap().rearrange("(k p) e -> p k e", p=HD)
    wo_ap = wo_d.ap().rearrange("(h p) e -> p h e", p=HD)
    EXP = mybir.ActivationFunctionType.Exp

    with tile.TileContext(nc) as tc, ExitStack() as top:
        constp = top.enter_context(tc.tile_pool(name="const", bufs=1))
        ones_sb = constp.tile([HD, 1], BF)
        nc.scalar.dma_start(ones_sb[:], ones_d[:])
        csp = top.enter_context(tc.tile_pool(name="csp", bufs=1))
        cs_sb = csp.tile([HD, 2, T], BF)

        qkp = top.enter_context(tc.tile_pool(name="qkp", bufs=1))
        # q heads at [:, h, :], k heads at [:, 4+h, :]
        qk_sb = qkp.tile([HD, 2 * HPC, T], BF)

        # x^T resident for both the q/k pass and the v pass
        with ExitStack() as vph:
            xtp = vph.enter_context(tc.tile_pool(name="xtp", side="right", bufs=1))
            xt_sb = xtp.tile([HD, NCHUNK, T], BF)
            # slice 0 first so the first matmul chain can start; the
            # cos/sin table is not needed until the first RoPE ~14us in
            nc.sync.dma_start(xt_sb[:, :, 0:SL], xt_ap[:, :, 0:SL])
            nc.sync.dma_start(cs_sb[:], cs_d[:])
            for ns in range(1, NSL):
                tsl = slice(ns * SL, (ns + 1) * SL)
                nc.sync.dma_start(xt_sb[:, :, tsl], xt_ap[:, :, tsl])
            wvp = vph.enter_context(tc.tile_pool(name="wvp", side="right", bufs=1))
            wv_sb = wvp.tile([HD, NCHUNK, HPC * HD], BF)

            # ---- combined q/k projection pass (+ fused RoPE) ----
            with ExitStack() as ph:
                wp = ph.enter_context(tc.tile_pool(name="wp", side="right", bufs=1))
                rawp = ph.enter_context(tc.tile_pool(name="rawp", side="right", bufs=2))
                swp = ph.enter_context(tc.tile_pool(name="swp", side="right", bufs=2))
                tbp = ph.enter_context(tc.tile_pool(name="tbp", side="right", bufs=2))
                pps = ph.enter_context(tc.tile_pool(name="pps", bufs=2, space="PSUM"))
                wqk_sb = wp.tile([HD, 2, NCHUNK, HPC * HD], BF)
                # weights on the scalar queue, split per k-chunk so the
                # first matmuls start as soon as chunk 0 lands
                for k in range(NCHUNK):
                    nc.scalar.dma_start(wqk_sb[:, 0, k, :], wq_ap[:, k, :])
                for k in range(NCHUNK):
                    nc.scalar.dma_start(wqk_sb[:, 1, k, :], wk_ap[:, k, :])
                for k in range(NCHUNK):  # prefetch wv for the next pass
                    nc.scalar.dma_start(wv_sb[:, k, :], wv_ap[:, k, :])
                for ns in range(NSL):
                    tsl = slice(ns * SL, (ns + 1) * SL)
                    for wsel in range(2):
                        ps = pps.tile([HD, HPC, SL], F32, tag="ps")
                        for h in range(HPC):
                            hs = slice(h * HD, (h + 1) * HD)
                            for k in range(NCHUNK):
                                nc.tensor.matmul(
                                    ps[:, h, :],
                                    wqk_sb[:, wsel, k, hs],
                                    xt_sb[:, k, tsl],
                                    start=(k == 0),
                                    stop=(k == NCHUNK - 1),
                                )
                        # Slab eviction: raw copy + partition-half swap,
                        # three big scalar copies (the ACT engine is the
                        # only one that can cross partitions cheaply).
                        raw = rawp.tile([HD, HPC, SL], BF, tag="raw")
                        sw = swp.tile([HD, HPC, SL], BF, tag="sw")
                        nc.scalar.copy(raw[:], ps[:])
                        nc.scalar.copy(sw[0:64, :, :], ps[64:128, :, :])
                        nc.scalar.copy(sw[64:128, :, :], ps[0:64, :, :])
                        # RoPE per head on the DVE at the bf16 2x rate:
                        # dst = raw*C + sw*S with C=[cos;cos], S=[-sin;sin]
                        for h in range(HPC):
                            dst = qk_sb[:, wsel * HPC + h, tsl]
                            tb = tbp.tile([HD, SL], BF, tag="tb")
                            nc.vector.tensor_mul(dst, raw[:, h, :], cs_sb[:, 0, tsl])
                            nc.vector.tensor_mul(tb[:], sw[:, h, :], cs_sb[:, 1, tsl])
                            nc.vector.tensor_add(dst, dst, tb[:])

            # ---- v projection (normal layout, x^T stationary) ----
            vap = top.enter_context(tc.tile_pool(name="vap", bufs=1))
            v_all = vap.tile([HD, NCHUNK, HPC * HD], BF)
            with ExitStack() as ph:
                vps = ph.enter_context(tc.tile_pool(name="vps", bufs=4, space="PSUM"))
                # descending chunk order so the attention pass (which
                # starts at the last query group) finds its first v
                # chunks ready immediately
                for c in reversed(range(NCHUNK)):
                    tcs = slice(c * HD, (c + 1) * HD)
                    ps = vps.tile([HD, HPC * HD], F32, tag="vps")
                    for k in range(NCHUNK):
                        nc.tensor.matmul(
                            ps[:],
                            xt_sb[:, k, tcs],
                            wv_sb[:, k, :],
                            start=(k == 0),
                            stop=(k == NCHUNK - 1),
                        )
                    nc.scalar.copy(v_all[:, c, :], ps[:])

        # ---- attention with interleaved output projection ----
        ctxp = top.enter_context(tc.tile_pool(name="ctxp", bufs=1))
        ctx_sb = ctxp.tile([HD, HPC, T], BF)
        wop = top.enter_context(tc.tile_pool(name="wop", bufs=1))
        wo_sb = wop.tile([HD, HPC, D], BF)
        with ExitStack() as ph:
            ptp = ph.enter_context(tc.tile_pool(name="ptp", side="right", bufs=2))
            mkpre = ph.enter_context(tc.tile_pool(name="mkpre", side="right", bufs=1))
            lrp = ph.enter_context(tc.tile_pool(name="lrp", side="right", bufs=2))
            rbp = ph.enter_context(tc.tile_pool(name="rbp", side="right", bufs=2))
            evp = ph.enter_context(tc.tile_pool(name="evp", side="right", bufs=2))
            sps = ph.enter_context(tc.tile_pool(name="sps", bufs=2, space="PSUM"))
            ops = ph.enter_context(tc.tile_pool(name="ops", bufs=2, space="PSUM"))
            wops = ph.enter_context(tc.tile_pool(name="wops", bufs=1, space="PSUM"))

            mk_sb = mkpre.tile([HD, nmask, PAIR], BF)
            nc.gpsimd.dma_start(mk_sb[:], mk_d[:])
            for h in range(HPC):  # prefetch wo on the idle gpsimd queue
                nc.gpsimd.dma_start(wo_sb[:, h, :], wo_ap[:, h, :])

            def finalize(st):
                # off the tensor engine: DVE fast-recip -> GpSimd partition
                # broadcast -> DVE multiply into ctx
                lr = lrp.tile([1, PAIR], F32, tag="lr")
                nc.vector.reciprocal_approx_fast(lr[:], st["l"])
                rb_sb = rbp.tile([HD, PAIR], F32, tag="rb")
                nc.gpsimd.partition_broadcast(rb_sb[:], lr[:])
                nc.vector.tensor_mul(
                    ctx_sb[:, st["h"], st["qsl"]], st["o"], rb_sb[:]
                )

            # outproj work items: one PSUM tile = 2 e-slices of 512 for
            # one 128-row t-chunk, contracted over the 4 local heads
            op_queue = []

            def push_outproj(j):
                for tck in (2 * j, 2 * j + 1):
                    for ep in range(2):
                        op_queue.append((tck, ep))

            def emit_outproj():
                if not op_queue:
                    return
                tck, ep = op_queue.pop(0)
                tsl = slice(tck * HD, (tck + 1) * HD)
                ps = wops.tile([HD, 2, SL], F32, tag="wo")
                for e2 in range(2):
                    esl = slice((ep * 2 + e2) * SL, (ep * 2 + e2 + 1) * SL)
                    for h in range(HPC):
                        nc.tensor.matmul(
                            ps[:, e2, :],
                            ctx_sb[:, h, tsl],
                            wo_sb[:, h, esl],
                            start=(h == 0),
                            stop=(h == HPC - 1),
                        )
                ev = evp.tile([HD, 2, SL], BF, tag="ev")
                nc.vector.tensor_copy(ev[:], ps[:])
                nc.sync.dma_start(
                    out_d[tsl, ep * 2 * SL:(ep + 1) * 2 * SL], ev[:]
                )

            def emit_ol(dq):
                # deferred p@v matmuls for an exp'd quad.  o and l share
                # one PSUM bank, so the l chain only starts after the o
                # group has closed (two accumulation groups open in the
                # same bank corrupt each other).
                pi, quad, st = dq
                h = st["h"]
                for t, (c, mi) in enumerate(quad):
                    nc.tensor.matmul(
                        st["o"],
                        v_all[:, c, h * HD:(h + 1) * HD],
                        st["pt"][:, pi + t, :],
                        start=(st["oi"] == 0),
                        stop=(st["oi"] == st["n"] - 1),
                        skip_group_check=True,
                    )
                    st["oi"] += 1
                if st["oi"] < st["n"]:
                    return False
                for idx in range(st["n"]):
                    nc.tensor.matmul(
                        st["l"],
                        ones_sb[:],
                        st["pt"][:, idx, :],
                        start=(idx == 0),
                        stop=(idx == st["n"] - 1),
                        skip_group_check=True,
                    )
                return True

            pending_ol = None
            pending_fin = None
            for j in reversed(range(NPAIR)):
                qsl = slice(j * PAIR, (j + 1) * PAIR)
                chunks = list(reversed(statuses[j]))
                n = len(chunks)
                quads = [chunks[ii:ii + 4] for ii in range(0, n, 4)]
                for h in range(HPC):
                    o_l = ops.tile([HD, 2, PAIR], F32, tag="o")
                    pt = ptp.tile([HD, NCHUNK, PAIR], BF, tag="pt")
                    st = {"o": o_l[:, 0, :], "l": o_l[0:1, 1, :],
                          "pt": pt, "h": h, "qsl": qsl, "n": n,
                          "oi": 0, "li": 0}
                    for qi, quad in enumerate(quads):
                        w = len(quad)
                        s_ps = sps.tile([HD, 4, PAIR], F32, tag="s")
                        for t, (c, mi) in enumerate(quad):
                            nc.tensor.matmul(
                                s_ps[:, t, :],
                                qk_sb[:, HPC + h, c * HD:(c + 1) * HD],
                                qk_sb[:, h, qsl],
                                start=True,
                                stop=True,
                            )
                        nc.scalar.activation(
                            pt[:, qi * 4:qi * 4 + w, :], s_ps[:, 0:w, :], EXP
                        )
                        # multiplicative exp-mask applied to pt
                        # (exp(s+m) == exp(s)*exp(m)), off the exp chain
                        t = 0
                        while t < w:
                            c, mi = quad[t]
                            if mi < 0:
                                t += 1
                                continue
                            r = t + 1
                            while (r < w and quad[r][1] >= 0
                                   and quad[r][1] == quad[r - 1][1] + 1):
                                r += 1
                            sl = slice(qi * 4 + t, qi * 4 + r)
                            nc.vector.tensor_mul(
                                pt[:, sl, :], pt[:, sl, :],
                                mk_sb[:, mi:mi + (r - t), :],
                            )
                            t = r
                        if pending_ol is not None:
                            if emit_ol(pending_ol):
                                pending_fin = pending_ol[2]
                            pending_ol = None
                        if pending_fin is not None and pending_fin is not st:
                            fj = pending_fin["qsl"].start // PAIR
                            fh = pending_fin["h"]
                            finalize(pending_fin)
                            pending_fin = None
                            if fh == HPC - 1:
                                push_outproj(fj)
                            emit_outproj()
                            if fj == 0 or (fj == 1 and fh == HPC - 1):
                                emit_outproj()
                        pending_ol = (qi * 4, quad, st)
            if pending_ol is not None:
                if emit_ol(pending_ol):
                    pending_fin = pending_ol[2]
            if pending_fin is not None:
                finalize(pending_fin)
                push_outproj(0)
            while op_queue:
                emit_outproj()
    nc.compile()
    return nc


_PERM = np.concatenate(
    [np.concatenate([np.arange(0, HD, 2), np.arange(1, HD, 2)]) + h * HD
     for h in range(HPC)]
)


def _bf16(a):
    import ml_dtypes

    return np.asarray(a, np.float32).astype(ml_dtypes.bfloat16)


def prepare(x, freqs, mask, wq, wk, wv, wo):
    """Host-side sharding/prep. Returns (nc, in_maps)."""
    x = np.asarray(x, np.float32)
    freqs = np.asarray(freqs, np.float32)
    mask = np.asarray(mask, np.float32)
    wq, wk, wv, wo = (np.asarray(w, np.float32) for w in (wq, wk, wv, wo))

    statuses, maskt = _mask_structure(mask)
    nc = _build_program(statuses, maskt.shape[1])

    scale = np.float32(1.0 / np.sqrt(HD))
    cos = np.ascontiguousarray(freqs[:, :, 0].T)  # (64, T)
    sin = np.ascontiguousarray(freqs[:, :, 1].T)
    cs = np.empty((HD, 2, T), np.float32)
    cs[0:64, 0, :] = cos
    cs[64:128, 0, :] = cos
    cs[0:64, 1, :] = -sin
    cs[64:128, 1, :] = sin

    ones_col = np.ones((HD, 1), np.float32)
    xt = [_bf16(x[b].T) for b in range(B)]
    cs_b = _bf16(cs)
    mk_b = _bf16(maskt)
    ones_b = _bf16(ones_col)

    in_maps = []
    for core in range(8):
        b, g = core // 4, core % 4
        cols = slice(g * HPC * HD, (g + 1) * HPC * HD)
        in_maps.append({
            "xt": xt[b],
            "wqt": _bf16((wq.T[:, cols] * scale)[:, _PERM]),
            "wkt": _bf16(wk.T[:, cols][:, _PERM]),
            "wvt": _bf16(wv.T[:, cols]),
            "wot": _bf16(wo.T[cols, :]),
            "cs": cs_b,
            "maskt": mk_b,
            "ones_col": ones_b,
        })
    return nc, in_maps


def run(x, freqs, mask, wq, wk, wv, wo, **spmd_kwargs):
    nc, in_maps = prepare(x, freqs, mask, wq, wk, wv, wo)
    res = run_bass_kernel_spmd(nc, in_maps, list(range(8)), **spmd_kwargs)
    parts = [res.results[c]["out"].astype(np.float32) for c in range(8)]
    out = np.stack([
        parts[b * 4] + parts[b * 4 + 1] + parts[b * 4 + 2] + parts[b * 4 + 3]
        for b in range(B)
    ]).astype(np.float32)
    return out, res


def kernel(x, freqs, mask, wq, wk, wv, wo):
    out, _ = run(x, freqs, mask, wq, wk, wv, wo)
    return out


# revision 11
# speedup vs baseline: 1.1372x; 1.0066x over previous
"""Trainium2 Bass kernel for causal self-attention with RoPE.

Shapes: x (2, 2048, 2048), 16 heads x 128 head_dim.
Sharding: 8 cores = 2 batch x 4 head-groups (4 heads per core).
Each core computes q/k/v projections for its heads, RoPE, causal-masked
softmax attention, and a partial output projection (its head columns of
wo); the host sums the 4 partials per batch element.

All matmul operands are bf16 (PSUM accumulation stays fp32): bf16 and
fp32r stream at the same 1 cycle/row on the PE, but bf16 halves DMA
traffic and SBUF footprint, loads stationary weights at 1 cycle/row,
and unlocks the DVE 2x packed mode for the element-wise work.

Layout strategy (per core):
  - q,k built in transposed layout (head_dim on partitions, t free) so
    RoPE and the score matmuls need no on-device transposes.  The host
    permutes wq/wk columns so RoPE's even/odd pairs become the two
    partition halves, and pre-scales wq by 1/sqrt(head_dim).
  - projections run in 512-query slices into a 4-bank PSUM slab (one
    bank per head); eviction+pair-swap amortize into three big scalar
    copies per slab, then RoPE is two DVE multiplies and an add per
    head at the 2x bf16 rate.
  - x^T stays resident in SBUF for the whole kernel (bf16 makes it
    fit), so the v pass reuses it without a second HBM read.
  - scores computed as s^T (keys x q) per 256-query group; softmax
    skips the max-subtraction (scores are O(1) by construction); row
    sums via a ones-vector matmul (the PE is the cheapest engine for a
    partition-axis reduction); normalization folded into the PSUM
    eviction on the DVE.
  - fully-masked key blocks are skipped (host inspects the mask);
    deduplicated exp(mask) tiles multiply pt only where a block is
    partially masked.
  - the output projection is interleaved into the attention loop (one
    PSUM group popped after each head finalize), so its matmuls fill
    the PE during softmax dependency stalls instead of forming a
    serial tail, and the tensor engine never idles long enough to
    lose its p-state.
"""

import sys
from contextlib import ExitStack

if "/opt/trn_rl_repo" not in sys.path:
    sys.path.insert(0, "/opt/trn_rl_repo")

import numpy as np

import concourse.bacc as bacc
import concourse.mybir as mybir
import concourse.tile as tile
from concourse.bass_utils import run_bass_kernel_spmd

B, T, D, NH, HD = 2, 2048, 2048, 16, 128
HPC = 4              # heads per core
SL = 512             # projection slice width (max moving dim)
NSL = T // SL        # 4
PAIR = 256           # queries per attention group
NPAIR = T // PAIR    # 8
NCHUNK = T // HD     # 16 key chunks of 128
BF = mybir.dt.bfloat16
F32 = mybir.dt.float32


def _mask_structure(mask):
    """Classify each (query-group, key-chunk) block of the additive mask.

    Returns (statuses, maskt): statuses[j] is a list of
    (chunk, mask_tile_index_or_minus1) for blocks that must be computed;
    maskt is the packed (128, nmask, 256) array of deduplicated
    transposed exp(mask) tiles for partially-masked blocks.
    """
    statuses = []
    tiles = {}
    tile_list = []
    for j in range(NPAIR):
        q = slice(j * PAIR, (j + 1) * PAIR)
        lst = []
        for c in range(NCHUNK):
            k = slice(c * HD, (c + 1) * HD)
            sub = mask[q, k]
            if np.all(sub <= -1e8):
                continue
            if np.all(sub == 0.0):
                lst.append((c, -1))
            else:
                key = sub.tobytes()
                mi = tiles.get(key)
                if mi is None:
                    mi = len(tile_list)
                    tiles[key] = mi
                    tile_list.append(np.ascontiguousarray(sub.T))
                lst.append((c, mi))
        assert lst, f"query group {j} has every key block masked"
        statuses.append(lst)
    nmask = max(1, len(tile_list))
    assert nmask <= 24, "too many distinct mask tiles to preload"
    maskt = np.zeros((HD, nmask, PAIR), np.float32)
    for i, t in enumerate(tile_list):
        assert np.all(t <= 64.0), "additive mask too large for exp-mask trick"
        maskt[:, i, :] = np.exp(t)
    return statuses, maskt


def _build_program(statuses, nmask):
    nc = bacc.Bacc(None, target_bir_lowering=False)

    xt_d = nc.dram_tensor("xt", [D, T], BF, kind="ExternalInput")
    wq_d = nc.dram_tensor("wqt", [D, HPC * HD], BF, kind="ExternalInput")
    wk_d = nc.dram_tensor("wkt", [D, HPC * HD], BF, kind="ExternalInput")
    wv_d = nc.dram_tensor("wvt", [D, HPC * HD], BF, kind="ExternalInput")
    wo_d = nc.dram_tensor("wot", [HPC * HD, D], BF, kind="ExternalInput")
    cs_d = nc.dram_tensor("cs", [HD, 2, T], BF, kind="ExternalInput")
    mk_d = nc.dram_tensor("maskt", [HD, nmask, PAIR], BF, kind="ExternalInput")
    ones_d = nc.dram_tensor("ones_col", [HD, 1], BF, kind="ExternalInput")
    out_d = nc.dram_tensor("out", [T, D], BF, kind="ExternalOutput")

    xt_ap = xt_d.ap().rearrange("(k p) t -> p k t", p=HD)
    wq_ap = wq_d.ap().rearrange("(k p) e -> p k e", p=HD)
    wk_ap = wk_d.ap().rearrange("(k p) e -> p k e", p=HD)
    wv_ap = wv_d.ap().rearrange("(k p) e -> p k e", p=HD)
    wo_ap = wo_d.ap().rearrange("(h p) e -> p h e", p=HD)
    EXP = mybir.ActivationFunctionType.Exp

    with tile.TileContext(nc) as tc, ExitStack() as top:
        constp = top.enter_context(tc.tile_pool(name="const", bufs=1))
        ones_sb = constp.tile([HD, 1], BF)
        nc.scalar.dma_start(ones_sb[:], ones_d[:])
        csp = top.enter_context(tc.tile_pool(name="csp", bufs=1))
        cs_sb = csp.tile([HD, 2, T], BF)

        qkp = top.enter_context(tc.tile_pool(name="qkp", bufs=1))
        # q heads at [:, h, :], k heads at [:, 4+h, :]
        qk_sb = qkp.tile([HD, 2 * HPC, T], BF)

        # x^T resident for both the q/k pass and the v pass
        with ExitStack() as vph:
            xtp = vph.enter_context(tc.tile_pool(name="xtp", side="right", bufs=1))
            xt_sb = xtp.tile([HD, NCHUNK, T], BF)
            # the q/k pass runs slices high-to-low (attention starts at
            # the last query group), so load the last slice first; the
            # cos/sin table is not needed until the first RoPE ~14us in
            nc.sync.dma_start(
                xt_sb[:, :, T - SL:T], xt_ap[:, :, T - SL:T]
            )
            nc.sync.dma_start(cs_sb[:], cs_d[:])
            for ns in reversed(range(NSL - 1)):
                tsl = slice(ns * SL, (ns + 1) * SL)
                nc.sync.dma_start(xt_sb[:, :, tsl], xt_ap[:, :, tsl])
            wvp = vph.enter_context(tc.tile_pool(name="wvp", side="right", bufs=1))
            wv_sb = wvp.tile([HD, NCHUNK, HPC * HD], BF)

            # ---- combined q/k projection pass (+ fused RoPE) ----
            with ExitStack() as ph:
                wp = ph.enter_context(tc.tile_pool(name="wp", side="right", bufs=1))
                rawp = ph.enter_context(tc.tile_pool(name="rawp", side="right", bufs=2))
                swp = ph.enter_context(tc.tile_pool(name="swp", side="right", bufs=2))
                tbp = ph.enter_context(tc.tile_pool(name="tbp", side="right", bufs=2))
                pps = ph.enter_context(tc.tile_pool(name="pps", bufs=2, space="PSUM"))
                wqk_sb = wp.tile([HD, 2, NCHUNK, HPC * HD], BF)
                # weights on the scalar queue, split per k-chunk so the
                # first matmuls start as soon as chunk 0 lands
                for k in range(NCHUNK):
                    nc.scalar.dma_start(wqk_sb[:, 0, k, :], wq_ap[:, k, :])
                for k in range(NCHUNK):  # wk on the idle gpsimd queue
                    nc.gpsimd.dma_start(wqk_sb[:, 1, k, :], wk_ap[:, k, :])
                for k in range(NCHUNK):  # prefetch wv for the next pass
                    nc.gpsimd.dma_start(wv_sb[:, k, :], wv_ap[:, k, :])
                for ns in reversed(range(NSL)):
                    tsl = slice(ns * SL, (ns + 1) * SL)
                    for wsel in range(2):
                        ps = pps.tile([HD, HPC, SL], F32, tag="ps")
                        for h in range(HPC):
                            hs = slice(h * HD, (h + 1) * HD)
                            for k in range(NCHUNK):
                                nc.tensor.matmul(
                                    ps[:, h, :],
                                    wqk_sb[:, wsel, k, hs],
                                    xt_sb[:, k, tsl],
                                    start=(k == 0),
                                    stop=(k == NCHUNK - 1),
                                )
                        # Slab eviction: raw copy + partition-half swap,
                        # three big scalar copies (the ACT engine is the
                        # only one that can cross partitions cheaply).
                        raw = rawp.tile([HD, HPC, SL], BF, tag="raw")
                        sw = swp.tile([HD, HPC, SL], BF, tag="sw")
                        nc.scalar.copy(raw[:], ps[:])
                        nc.scalar.copy(sw[0:64, :, :], ps[64:128, :, :])
                        nc.scalar.copy(sw[64:128, :, :], ps[0:64, :, :])
                        # RoPE per head on the DVE at the bf16 2x rate:
                        # dst = raw*C + sw*S with C=[cos;cos], S=[-sin;sin]
                        for h in range(HPC):
                            dst = qk_sb[:, wsel * HPC + h, tsl]
                            tb = tbp.tile([HD, SL], BF, tag="tb")
                            nc.vector.tensor_mul(dst, raw[:, h, :], cs_sb[:, 0, tsl])
                            nc.vector.tensor_mul(tb[:], sw[:, h, :], cs_sb[:, 1, tsl])
                            nc.vector.tensor_add(dst, dst, tb[:])

            # ---- v projection (normal layout, x^T stationary) ----
            vap = top.enter_context(tc.tile_pool(name="vap", bufs=1))
            v_all = vap.tile([HD, NCHUNK, HPC * HD], BF)
            with ExitStack() as ph:
                vps = ph.enter_context(tc.tile_pool(name="vps", bufs=4, space="PSUM"))
                # descending chunk order so the attention pass (which
                # starts at the last query group) finds its first v
                # chunks ready immediately
                for c in reversed(range(NCHUNK)):
                    tcs = slice(c * HD, (c + 1) * HD)
                    ps = vps.tile([HD, HPC * HD], F32, tag="vps")
                    for k in range(NCHUNK):
                        nc.tensor.matmul(
                            ps[:],
                            xt_sb[:, k, tcs],
                            wv_sb[:, k, :],
                            start=(k == 0),
                            stop=(k == NCHUNK - 1),
                        )
                    nc.scalar.copy(v_all[:, c, :], ps[:])

        # ---- attention with interleaved output projection ----
        ctxp = top.enter_context(tc.tile_pool(name="ctxp", bufs=1))
        ctx_sb = ctxp.tile([HD, HPC, T], BF)
        wop = top.enter_context(tc.tile_pool(name="wop", bufs=1))
        wo_sb = wop.tile([HD, HPC, D], BF)
        with ExitStack() as ph:
            ptp = ph.enter_context(tc.tile_pool(name="ptp", side="right", bufs=2))
            mkpre = ph.enter_context(tc.tile_pool(name="mkpre", side="right", bufs=1))
            lrp = ph.enter_context(tc.tile_pool(name="lrp", side="right", bufs=2))
            rbp = ph.enter_context(tc.tile_pool(name="rbp", side="right", bufs=2))
            evp = ph.enter_context(tc.tile_pool(name="evp", side="right", bufs=2))
            sps = ph.enter_context(tc.tile_pool(name="sps", bufs=2, space="PSUM"))
            ops = ph.enter_context(tc.tile_pool(name="ops", bufs=2, space="PSUM"))
            wops = ph.enter_context(tc.tile_pool(name="wops", bufs=2, space="PSUM"))

            mk_sb = mkpre.tile([HD, nmask, PAIR], BF)
            nc.gpsimd.dma_start(mk_sb[:], mk_d[:])
            for h in range(HPC):  # prefetch wo on the idle gpsimd queue
                nc.gpsimd.dma_start(wo_sb[:, h, :], wo_ap[:, h, :])

            def finalize(st):
                # off the tensor engine: DVE fast-recip -> GpSimd partition
                # broadcast -> DVE multiply into ctx
                lr = lrp.tile([1, PAIR], F32, tag="lr")
                nc.vector.reciprocal_approx_fast(lr[:], st["l"])
                rb_sb = rbp.tile([HD, PAIR], F32, tag="rb")
                nc.gpsimd.partition_broadcast(rb_sb[:], lr[:])
                nc.vector.tensor_mul(
                    ctx_sb[:, st["h"], st["qsl"]], st["o"], rb_sb[:]
                )

            # outproj work items: one PSUM bank = one e-slice of 512 for
            # one 128-row t-chunk, contracted over the 4 local heads
            op_queue = []

            def push_outproj(j):
                for tck in (2 * j, 2 * j + 1):
                    for es in range(4):
                        op_queue.append((tck, es))

            def emit_outproj():
                if not op_queue:
                    return
                tck, es = op_queue.pop(0)
                tsl = slice(tck * HD, (tck + 1) * HD)
                esl = slice(es * SL, (es + 1) * SL)
                ps = wops.tile([HD, SL], F32, tag="wo")
                for h in range(HPC):
                    nc.tensor.matmul(
                        ps[:],
                        ctx_sb[:, h, tsl],
                        wo_sb[:, h, esl],
                        start=(h == 0),
                        stop=(h == HPC - 1),
                    )
                ev = evp.tile([HD, SL], BF, tag="ev")
                nc.vector.tensor_copy(ev[:], ps[:])
                nc.sync.dma_start(out_d[tsl, esl], ev[:])

            def emit_ol(dq):
                # deferred p@v matmuls for an exp'd quad.  o and l share
                # one PSUM bank, so the l chain only starts after the o
                # group has closed (two accumulation groups open in the
                # same bank corrupt each other).
                pi, quad, st = dq
                h = st["h"]
                for t, (c, mi) in enumerate(quad):
                    nc.tensor.matmul(
                        st["o"],
                        v_all[:, c, h * HD:(h + 1) * HD],
                        st["pt"][:, pi + t, :],
                        start=(st["oi"] == 0),
                        stop=(st["oi"] == st["n"] - 1),
                        skip_group_check=True,
                    )
                    st["oi"] += 1
                if st["oi"] < st["n"]:
                    return False
                for idx in range(st["n"]):
                    nc.tensor.matmul(
                        st["l"],
                        ones_sb[:],
                        st["pt"][:, idx, :],
                        start=(idx == 0),
                        stop=(idx == st["n"] - 1),
                        skip_group_check=True,
                    )
                return True

            pending_ol = None
            pending_fin = None
            for j in reversed(range(NPAIR)):
                qsl = slice(j * PAIR, (j + 1) * PAIR)
                chunks = list(reversed(statuses[j]))
                n = len(chunks)
                quads = [chunks[ii:ii + 4] for ii in range(0, n, 4)]
                for h in range(HPC):
                    o_l = ops.tile([HD, 2, PAIR], F32, tag="o")
                    pt = ptp.tile([HD, NCHUNK, PAIR], BF, tag="pt")
                    st = {"o": o_l[:, 0, :], "l": o_l[0:1, 1, :],
                          "pt": pt, "h": h, "qsl": qsl, "n": n,
                          "oi": 0, "li": 0}
                    for qi, quad in enumerate(quads):
                        w = len(quad)
                        s_ps = sps.tile([HD, 4, PAIR], F32, tag="s")
                        for t, (c, mi) in enumerate(quad):
                            nc.tensor.matmul(
                                s_ps[:, t, :],
                                qk_sb[:, HPC + h, c * HD:(c + 1) * HD],
                                qk_sb[:, h, qsl],
                                start=True,
                                stop=True,
                            )
                        nc.scalar.activation(
                            pt[:, qi * 4:qi * 4 + w, :], s_ps[:, 0:w, :], EXP
                        )
                        # multiplicative exp-mask applied to pt
                        # (exp(s+m) == exp(s)*exp(m)), off the exp chain
                        t = 0
                        while t < w:
                            c, mi = quad[t]
                            if mi < 0:
                                t += 1
                                continue
                            r = t + 1
                            while (r < w and quad[r][1] >= 0
                                   and quad[r][1] == quad[r - 1][1] + 1):
                                r += 1
                            sl = slice(qi * 4 + t, qi * 4 + r)
                            nc.vector.tensor_mul(
                                pt[:, sl, :], pt[:, sl, :],
                                mk_sb[:, mi:mi + (r - t), :],
                            )
                            t = r
                        if pending_ol is not None:
                            if emit_ol(pending_ol):
                                pending_fin = pending_ol[2]
                            pending_ol = None
                        if pending_fin is not None and pending_fin is not st:
                            fj = pending_fin["qsl"].start // PAIR
                            fh = pending_fin["h"]
                            finalize(pending_fin)
                            pending_fin = None
                            if fh == HPC - 1:
                                push_outproj(fj)
                            emit_outproj()
                            emit_outproj()
                            if fj <= 1:
                                emit_outproj()
                        pending_ol = (qi * 4, quad, st)
            if pending_ol is not None:
                if emit_ol(pending_ol):
                    pending_fin = pending_ol[2]
            if pending_fin is not None:
                finalize(pending_fin)
                push_outproj(0)
            while op_queue:
                emit_outproj()
    nc.compile()
    return nc


_PERM = np.concatenate(
    [np.concatenate([np.arange(0, HD, 2), np.arange(1, HD, 2)]) + h * HD
     for h in range(HPC)]
)


def _bf16(a):
    import ml_dtypes

    return np.asarray(a, np.float32).astype(ml_dtypes.bfloat16)


def prepare(x, freqs, mask, wq, wk, wv, wo):
    """Host-side sharding/prep. Returns (nc, in_maps)."""
    x = np.asarray(x, np.float32)
    freqs = np.asarray(freqs, np.float32)
    mask = np.asarray(mask, np.float32)
    wq, wk, wv, wo = (np.asarray(w, np.float32) for w in (wq, wk, wv, wo))

    statuses, maskt = _mask_structure(mask)
    nc = _build_program(statuses, maskt.shape[1])

    scale = np.float32(1.0 / np.sqrt(HD))
    cos = np.ascontiguousarray(freqs[:, :, 0].T)  # (64, T)
    sin = np.ascontiguousarray(freqs[:, :, 1].T)
    cs = np.empty((HD, 2, T), np.float32)
    cs[0:64, 0, :] = cos
    cs[64:128, 0, :] = cos
    cs[0:64, 1, :] = -sin
    cs[64:128, 1, :] = sin

    ones_col = np.ones((HD, 1), np.float32)
    xt = [_bf16(x[b].T) for b in range(B)]
    cs_b = _bf16(cs)
    mk_b = _bf16(maskt)
    ones_b = _bf16(ones_col)

    in_maps = []
    for core in range(8):
        b, g = core // 4, core % 4
        cols = slice(g * HPC * HD, (g + 1) * HPC * HD)
        in_maps.append({
            "xt": xt[b],
            "wqt": _bf16((wq.T[:, cols] * scale)[:, _PERM]),
            "wkt": _bf16(wk.T[:, cols][:, _PERM]),
            "wvt": _bf16(wv.T[:, cols]),
            "wot": _bf16(wo.T[cols, :]),
            "cs": cs_b,
            "maskt": mk_b,
            "ones_col": ones_b,
        })
    return nc, in_maps


def run(x, freqs, mask, wq, wk, wv, wo, **spmd_kwargs):
    nc, in_maps = prepare(x, freqs, mask, wq, wk, wv, wo)
    res = run_bass_kernel_spmd(nc, in_maps, list(range(8)), **spmd_kwargs)
    parts = [res.results[c]["out"].astype(np.float32) for c in range(8)]
    out = np.stack([
        parts[b * 4] + parts[b * 4 + 1] + parts[b * 4 + 2] + parts[b * 4 + 3]
        for b in range(B)
    ]).astype(np.float32)
    return out, res


def kernel(x, freqs, mask, wq, wk, wv, wo):
    out, _ = run(x, freqs, mask, wq, wk, wv, wo)
    return out


# revision 12
# speedup vs baseline: 1.1439x; 1.0059x over previous
"""Trainium2 Bass kernel for causal self-attention with RoPE.

Shapes: x (2, 2048, 2048), 16 heads x 128 head_dim.
Sharding: 8 cores = 2 batch x 4 head-groups (4 heads per core).
Each core computes q/k/v projections for its heads, RoPE, causal-masked
softmax attention, and a partial output projection (its head columns of
wo); the host sums the 4 partials per batch element.

All matmul operands are bf16 (PSUM accumulation stays fp32): bf16 and
fp32r stream at the same 1 cycle/row on the PE, but bf16 halves DMA
traffic and SBUF footprint, loads stationary weights at 1 cycle/row,
and unlocks the DVE 2x packed mode for the element-wise work.

Layout strategy (per core):
  - all DRAM inputs are partition-major (the host pre-transposes), so
    every DMA moves >=16KB contiguous per partition instead of 1KB
    strided runs (which are descriptor-rate-bound at ~95GB/s).
  - q,k built in transposed layout (head_dim on partitions, t free) so
    RoPE and the score matmuls need no on-device transposes.  The host
    permutes wq/wk columns so RoPE's even/odd pairs become the two
    partition halves, and pre-scales wq by 1/sqrt(head_dim).
  - projections run in 512-query slices into a 4-bank PSUM slab (one
    bank per head); eviction+pair-swap amortize into three big scalar
    copies per slab, then RoPE is two DVE multiplies and an add per
    head at the 2x bf16 rate.
  - the v projection reuses the same PSUM slab pool (4 chunk-chains
    per slab, one big eviction copy), so there is no pool-close
    barrier stalling the PE between the q/k and v passes.
  - x^T stays resident in SBUF for the whole projection phase.
  - scores computed as s^T (keys x q) per 256-query group; softmax
    skips the max-subtraction (scores are O(1) by construction); row
    sums via a ones-vector matmul (the PE is the cheapest engine for a
    partition-axis reduction); normalization folded into the PSUM
    eviction on the DVE.  o and l share one PSUM bank; the l chain is
    deferred until the o group closes (two groups open in one bank
    corrupt each other).
  - fully-masked key blocks are skipped (host inspects the mask);
    deduplicated exp(mask) tiles multiply pt only where a block is
    partially masked.
  - the output projection is interleaved into the attention loop (one
    PSUM bank per 512-col group, popped after every score quad), so
    its matmuls fill the PE during softmax dependency stalls instead
    of forming a serial tail, and the tensor engine never idles long
    enough to lose its p-state.
"""

import sys
from contextlib import ExitStack

if "/opt/trn_rl_repo" not in sys.path:
    sys.path.insert(0, "/opt/trn_rl_repo")

import numpy as np

import concourse.bacc as bacc
import concourse.mybir as mybir
import concourse.tile as tile
from concourse.bass_utils import run_bass_kernel_spmd

B, T, D, NH, HD = 2, 2048, 2048, 16, 128
HPC = 4              # heads per core
SL = 512             # projection slice width (max moving dim)
NSL = T // SL        # 4
PAIR = 256           # queries per attention group
NPAIR = T // PAIR    # 8
NCHUNK = T // HD     # 16 key chunks of 128
BF = mybir.dt.bfloat16
F32 = mybir.dt.float32


def _mask_structure(mask):
    """Classify each (query-group, key-chunk) block of the additive mask.

    Returns (statuses, maskt): statuses[j] is a list of
    (chunk, mask_tile_index_or_minus1) for blocks that must be computed;
    maskt is the packed (128, nmask, 256) array of deduplicated
    transposed exp(mask) tiles for partially-masked blocks.
    """
    statuses = []
    tiles = {}
    tile_list = []
    for j in range(NPAIR):
        q = slice(j * PAIR, (j + 1) * PAIR)
        lst = []
        for c in range(NCHUNK):
            k = slice(c * HD, (c + 1) * HD)
            sub = mask[q, k]
            if np.all(sub <= -1e8):
                continue
            if np.all(sub == 0.0):
                lst.append((c, -1))
            else:
                key = sub.tobytes()
                mi = tiles.get(key)
                if mi is None:
                    mi = len(tile_list)
                    tiles[key] = mi
                    tile_list.append(np.ascontiguousarray(sub.T))
                lst.append((c, mi))
        assert lst, f"query group {j} has every key block masked"
        statuses.append(lst)
    nmask = max(1, len(tile_list))
    assert nmask <= 24, "too many distinct mask tiles to preload"
    maskt = np.zeros((HD, nmask, PAIR), np.float32)
    for i, t in enumerate(tile_list):
        assert np.all(t <= 64.0), "additive mask too large for exp-mask trick"
        maskt[:, i, :] = np.exp(t)
    return statuses, maskt


def _build_program(statuses, nmask):
    nc = bacc.Bacc(None, target_bir_lowering=False)

    # all inputs partition-major: leading dim = the 128 SBUF partitions
    xt_d = nc.dram_tensor("xts", [HD, NSL, NCHUNK, SL], BF, kind="ExternalInput")
    wq_d = nc.dram_tensor("wqt", [HD, NCHUNK, HPC * HD], BF, kind="ExternalInput")
    wk_d = nc.dram_tensor("wkt", [HD, NCHUNK, HPC * HD], BF, kind="ExternalInput")
    wv_d = nc.dram_tensor("wvt", [HD, NCHUNK, HPC * HD], BF, kind="ExternalInput")
    wo_d = nc.dram_tensor("wot", [HD, HPC, D], BF, kind="ExternalInput")
    cs_d = nc.dram_tensor("cs", [HD, 2, T], BF, kind="ExternalInput")
    mk_d = nc.dram_tensor("maskt", [HD, nmask, PAIR], BF, kind="ExternalInput")
    ones_d = nc.dram_tensor("ones_col", [HD, 1], BF, kind="ExternalInput")
    out_d = nc.dram_tensor("out", [T, D], BF, kind="ExternalOutput")

    EXP = mybir.ActivationFunctionType.Exp

    with tile.TileContext(nc) as tc, ExitStack() as top:
        constp = top.enter_context(tc.tile_pool(name="const", bufs=1))
        ones_sb = constp.tile([HD, 1], BF)
        nc.scalar.dma_start(ones_sb[:], ones_d[:])
        csp = top.enter_context(tc.tile_pool(name="csp", bufs=1))
        cs_sb = csp.tile([HD, 2, T], BF)

        qkp = top.enter_context(tc.tile_pool(name="qkp", bufs=1))
        # q heads at [:, h, :], k heads at [:, 4+h, :]
        qk_sb = qkp.tile([HD, 2 * HPC, T], BF)
        vap = top.enter_context(tc.tile_pool(name="vap", bufs=1))
        v_all = vap.tile([HD, NCHUNK, HPC * HD], BF)

        # ---- projection phase: q/k slabs (+RoPE), then v slabs ----
        with ExitStack() as ph:
            xtp = ph.enter_context(tc.tile_pool(name="xtp", side="right", bufs=1))
            xt_sb = xtp.tile([HD, NSL, NCHUNK, SL], BF)
            # the q/k pass runs slices high-to-low (attention starts at
            # the last query group), so load the last slice first, split
            # in two so the first chain can start sooner; the cos/sin
            # table is not needed until the first RoPE ~14us in
            nc.sync.dma_start(xt_sb[:, NSL - 1, 0:8, :], xt_d.ap()[:, NSL - 1, 0:8, :])
            nc.sync.dma_start(xt_sb[:, NSL - 1, 8:16, :], xt_d.ap()[:, NSL - 1, 8:16, :])
            nc.sync.dma_start(cs_sb[:], cs_d[:])
            for ns in reversed(range(NSL - 1)):
                nc.sync.dma_start(xt_sb[:, ns, :, :], xt_d.ap()[:, ns, :, :])

            wp = ph.enter_context(tc.tile_pool(name="wp", side="right", bufs=1))
            wvp = ph.enter_context(tc.tile_pool(name="wvp", side="right", bufs=1))
            rawp = ph.enter_context(tc.tile_pool(name="rawp", side="right", bufs=2))
            swp = ph.enter_context(tc.tile_pool(name="swp", side="right", bufs=2))
            tbp = ph.enter_context(tc.tile_pool(name="tbp", side="right", bufs=2))
            pps = ph.enter_context(tc.tile_pool(name="pps", bufs=2, space="PSUM"))
            wqk_sb = wp.tile([HD, 2, NCHUNK, HPC * HD], BF)
            wv_sb = wvp.tile([HD, NCHUNK, HPC * HD], BF)
            nc.scalar.dma_start(wqk_sb[:, 0, 0:8, :], wq_d.ap()[:, 0:8, :])
            nc.scalar.dma_start(wqk_sb[:, 0, 8:16, :], wq_d.ap()[:, 8:16, :])
            nc.gpsimd.dma_start(wqk_sb[:, 1, :, :], wk_d.ap()[:])
            nc.gpsimd.dma_start(wv_sb[:], wv_d.ap()[:])

            for ns in reversed(range(NSL)):
                tsl = slice(ns * SL, (ns + 1) * SL)
                for wsel in range(2):
                    ps = pps.tile([HD, HPC, SL], F32, tag="ps")
                    for h in range(HPC):
                        hs = slice(h * HD, (h + 1) * HD)
                        for k in range(NCHUNK):
                            nc.tensor.matmul(
                                ps[:, h, :],
                                wqk_sb[:, wsel, k, hs],
                                xt_sb[:, ns, k, :],
                                start=(k == 0),
                                stop=(k == NCHUNK - 1),
                            )
                    # Slab eviction: raw copy + partition-half swap,
                    # three big scalar copies (the ACT engine is the
                    # only one that can cross partitions cheaply).
                    raw = rawp.tile([HD, HPC, SL], BF, tag="raw")
                    sw = swp.tile([HD, HPC, SL], BF, tag="sw")
                    nc.scalar.copy(raw[:], ps[:])
                    nc.scalar.copy(sw[0:64, :, :], ps[64:128, :, :])
                    nc.scalar.copy(sw[64:128, :, :], ps[0:64, :, :])
                    # RoPE per head on the DVE at the bf16 2x rate:
                    # dst = raw*C + sw*S with C=[cos;cos], S=[-sin;sin]
                    for h in range(HPC):
                        dst = qk_sb[:, wsel * HPC + h, tsl]
                        tb = tbp.tile([HD, SL], BF, tag="tb")
                        nc.vector.tensor_mul(dst, raw[:, h, :], cs_sb[:, 0, tsl])
                        nc.vector.tensor_mul(tb[:], sw[:, h, :], cs_sb[:, 1, tsl])
                        nc.vector.tensor_add(dst, dst, tb[:])

            # v slabs from the same PSUM pool: descending chunk blocks
            # so the attention pass (which starts at the last query
            # group) finds its first v chunks ready immediately
            for c0 in (12, 8, 4, 0):
                ps = pps.tile([HD, HPC, SL], F32, tag="ps")
                for c in range(c0 + 3, c0 - 1, -1):
                    tcs = slice((c % 4) * HD, (c % 4 + 1) * HD)
                    for k in range(NCHUNK):
                        nc.tensor.matmul(
                            ps[:, c - c0, :],
                            xt_sb[:, c // 4, k, tcs],
                            wv_sb[:, k, :],
                            start=(k == 0),
                            stop=(k == NCHUNK - 1),
                        )
                nc.scalar.copy(v_all[:, c0:c0 + 4, :], ps[:])

        # ---- attention with interleaved output projection ----
        ctxp = top.enter_context(tc.tile_pool(name="ctxp", bufs=1))
        ctx_sb = ctxp.tile([HD, HPC, T], BF)
        wop = top.enter_context(tc.tile_pool(name="wop", bufs=1))
        wo_sb = wop.tile([HD, HPC, D], BF)
        with ExitStack() as ph:
            ptp = ph.enter_context(tc.tile_pool(name="ptp", side="right", bufs=2))
            mkpre = ph.enter_context(tc.tile_pool(name="mkpre", side="right", bufs=1))
            lrp = ph.enter_context(tc.tile_pool(name="lrp", side="right", bufs=2))
            rbp = ph.enter_context(tc.tile_pool(name="rbp", side="right", bufs=2))
            evp = ph.enter_context(tc.tile_pool(name="evp", side="right", bufs=3))
            sps = ph.enter_context(tc.tile_pool(name="sps", bufs=2, space="PSUM"))
            ops = ph.enter_context(tc.tile_pool(name="ops", bufs=2, space="PSUM"))
            wops = ph.enter_context(tc.tile_pool(name="wops", bufs=2, space="PSUM"))

            mk_sb = mkpre.tile([HD, nmask, PAIR], BF)
            nc.gpsimd.dma_start(mk_sb[:], mk_d[:])
            nc.gpsimd.dma_start(wo_sb[:], wo_d.ap()[:])

            def finalize(st):
                # off the tensor engine: DVE fast-recip -> GpSimd partition
                # broadcast -> DVE multiply into ctx
                lr = lrp.tile([1, PAIR], F32, tag="lr")
                nc.vector.reciprocal_approx_fast(lr[:], st["l"])
                rb_sb = rbp.tile([HD, PAIR], F32, tag="rb")
                nc.gpsimd.partition_broadcast(rb_sb[:], lr[:])
                nc.vector.tensor_mul(
                    ctx_sb[:, st["h"], st["qsl"]], st["o"], rb_sb[:]
                )

            # outproj work items: one PSUM bank = one e-slice of 512 for
            # one 128-row t-chunk, contracted over the 4 local heads
            op_queue = []

            def push_outproj(j):
                for tck in (2 * j, 2 * j + 1):
                    for es in range(4):
                        op_queue.append((tck, es))

            def emit_outproj(evict_scalar=False):
                if not op_queue:
                    return
                tck, es = op_queue.pop(0)
                tsl = slice(tck * HD, (tck + 1) * HD)
                esl = slice(es * SL, (es + 1) * SL)
                ps = wops.tile([HD, SL], F32, tag="wo")
                for h in range(HPC):
                    nc.tensor.matmul(
                        ps[:],
                        ctx_sb[:, h, tsl],
                        wo_sb[:, h, esl],
                        start=(h == 0),
                        stop=(h == HPC - 1),
                    )
                ev = evp.tile([HD, SL], BF, tag="ev")
                if evict_scalar:
                    nc.scalar.copy(ev[:], ps[:])
                else:
                    nc.vector.tensor_copy(ev[:], ps[:])
                nc.sync.dma_start(out_d[tsl, esl], ev[:])

            def emit_ol(dq):
                # deferred p@v matmuls for an exp'd quad.  o and l share
                # one PSUM bank, so the l chain only starts after the o
                # group has closed (two accumulation groups open in the
                # same bank corrupt each other).
                pi, quad, st = dq
                h = st["h"]
                for t, (c, mi) in enumerate(quad):
                    nc.tensor.matmul(
                        st["o"],
                        v_all[:, c, h * HD:(h + 1) * HD],
                        st["pt"][:, pi + t, :],
                        start=(st["oi"] == 0),
                        stop=(st["oi"] == st["n"] - 1),
                        skip_group_check=True,
                    )
                    st["oi"] += 1
                if st["oi"] < st["n"]:
                    return False
                for idx in range(st["n"]):
                    nc.tensor.matmul(
                        st["l"],
                        ones_sb[:],
                        st["pt"][:, idx, :],
                        start=(idx == 0),
                        stop=(idx == st["n"] - 1),
                        skip_group_check=True,
                    )
                return True

            pending_ol = None
            pending_fin = None
            for j in reversed(range(NPAIR)):
                qsl = slice(j * PAIR, (j + 1) * PAIR)
                chunks = list(reversed(statuses[j]))
                n = len(chunks)
                quads = [chunks[ii:ii + 4] for ii in range(0, n, 4)]
                for h in range(HPC):
                    o_l = ops.tile([HD, 2, PAIR], F32, tag="o")
                    pt = ptp.tile([HD, NCHUNK, PAIR], BF, tag="pt")
                    st = {"o": o_l[:, 0, :], "l": o_l[0:1, 1, :],
                          "pt": pt, "h": h, "qsl": qsl, "n": n,
                          "oi": 0}
                    for qi, quad in enumerate(quads):
                        w = len(quad)
                        s_ps = sps.tile([HD, 4, PAIR], F32, tag="s")
                        for t, (c, mi) in enumerate(quad):
                            nc.tensor.matmul(
                                s_ps[:, t, :],
                                qk_sb[:, HPC + h, c * HD:(c + 1) * HD],
                                qk_sb[:, h, qsl],
                                start=True,
                                stop=True,
                            )
                        nc.scalar.activation(
                            pt[:, qi * 4:qi * 4 + w, :], s_ps[:, 0:w, :], EXP
                        )
                        # multiplicative exp-mask applied to pt
                        # (exp(s+m) == exp(s)*exp(m)), off the exp chain
                        t = 0
                        while t < w:
                            c, mi = quad[t]
                            if mi < 0:
                                t += 1
                                continue
                            r = t + 1
                            while (r < w and quad[r][1] >= 0
                                   and quad[r][1] == quad[r - 1][1] + 1):
                                r += 1
                            sl = slice(qi * 4 + t, qi * 4 + r)
                            nc.vector.tensor_mul(
                                pt[:, sl, :], pt[:, sl, :],
                                mk_sb[:, mi:mi + (r - t), :],
                            )
                            t = r
                        if pending_ol is not None:
                            if emit_ol(pending_ol):
                                pending_fin = pending_ol[2]
                            pending_ol = None
                        if pending_fin is not None and pending_fin is not st:
                            fj = pending_fin["qsl"].start // PAIR
                            fh = pending_fin["h"]
                            finalize(pending_fin)
                            pending_fin = None
                            if fh == HPC - 1:
                                push_outproj(fj)
                        emit_outproj()
                        pending_ol = (qi * 4, quad, st)
            if pending_ol is not None:
                if emit_ol(pending_ol):
                    pending_fin = pending_ol[2]
            if pending_fin is not None:
                finalize(pending_fin)
                push_outproj(0)
            while op_queue:
                emit_outproj(evict_scalar=True)
    nc.compile()
    return nc


_PERM = np.concatenate(
    [np.concatenate([np.arange(0, HD, 2), np.arange(1, HD, 2)]) + h * HD
     for h in range(HPC)]
)


def _bf16(a):
    import ml_dtypes

    return np.ascontiguousarray(
        np.asarray(a, np.float32).astype(ml_dtypes.bfloat16)
    )


def _pmaj(w):
    # (D, E) weight -> partition-major (HD, NCHUNK_of_D, E)
    d, e = w.shape
    return w.reshape(d // HD, HD, e).transpose(1, 0, 2)


def prepare(x, freqs, mask, wq, wk, wv, wo):
    """Host-side sharding/prep. Returns (nc, in_maps)."""
    x = np.asarray(x, np.float32)
    freqs = np.asarray(freqs, np.float32)
    mask = np.asarray(mask, np.float32)
    wq, wk, wv, wo = (np.asarray(w, np.float32) for w in (wq, wk, wv, wo))

    statuses, maskt = _mask_structure(mask)
    nc = _build_program(statuses, maskt.shape[1])

    scale = np.float32(1.0 / np.sqrt(HD))
    cos = np.ascontiguousarray(freqs[:, :, 0].T)  # (64, T)
    sin = np.ascontiguousarray(freqs[:, :, 1].T)
    cs = np.empty((HD, 2, T), np.float32)
    cs[0:64, 0, :] = cos
    cs[64:128, 0, :] = cos
    cs[0:64, 1, :] = -sin
    cs[64:128, 1, :] = sin

    ones_col = np.ones((HD, 1), np.float32)
    # x^T in (HD, NSL, NCHUNK, SL) partition-major blocks:
    # element (p, s, k, t) = x[s*SL + t, k*HD + p]
    xts = [
        _bf16(x[b].T.reshape(NCHUNK, HD, NSL, SL).transpose(1, 2, 0, 3))
        for b in range(B)
    ]
    cs_b = _bf16(cs)
    mk_b = _bf16(maskt)
    ones_b = _bf16(ones_col)

    in_maps = []
    for core in range(8):
        b, g = core // 4, core % 4
        cols = slice(g * HPC * HD, (g + 1) * HPC * HD)
        in_maps.append({
            "xts": xts[b],
            "wqt": _bf16(_pmaj((wq.T[:, cols] * scale)[:, _PERM])),
            "wkt": _bf16(_pmaj(wk.T[:, cols][:, _PERM])),
            "wvt": _bf16(_pmaj(wv.T[:, cols])),
            "wot": _bf16(_pmaj(wo.T[cols, :])),
            "cs": cs_b,
            "maskt": mk_b,
            "ones_col": ones_b,
        })
    return nc, in_maps


def run(x, freqs, mask, wq, wk, wv, wo, **spmd_kwargs):
    nc, in_maps = prepare(x, freqs, mask, wq, wk, wv, wo)
    res = run_bass_kernel_spmd(nc, in_maps, list(range(8)), **spmd_kwargs)
    parts = [res.results[c]["out"].astype(np.float32) for c in range(8)]
    out = np.stack([
        parts[b * 4] + parts[b * 4 + 1] + parts[b * 4 + 2] + parts[b * 4 + 3]
        for b in range(B)
    ]).astype(np.float32)
    return out, res


def kernel(x, freqs, mask, wq, wk, wv, wo):
    out, _ = run(x, freqs, mask, wq, wk, wv, wo)
    return out


# revision 16
# speedup vs baseline: 1.1633x; 1.0170x over previous
"""Trainium2 Bass kernel for causal self-attention with RoPE.

Shapes: x (2, 2048, 2048), 16 heads x 128 head_dim.
Sharding: 8 cores = 2 batch x 4 head-groups (4 heads per core).
Each core computes q/k/v projections for its heads, RoPE, causal-masked
softmax attention, and a partial output projection (its head columns of
wo); the host sums the 4 partials per batch element.

All matmul operands are bf16 (PSUM accumulation stays fp32): bf16 and
fp32r stream at the same 1 cycle/row on the PE, but bf16 halves DMA
traffic and SBUF footprint, loads stationary weights at 1 cycle/row,
and unlocks the DVE 2x packed mode for the element-wise work.

Layout strategy (per core):
  - all DRAM inputs are partition-major (the host pre-transposes), so
    every DMA moves >=16KB contiguous per partition instead of 1KB
    strided runs (which are descriptor-rate-bound at ~95GB/s).
  - q,k built in transposed layout (head_dim on partitions, t free) so
    RoPE and the score matmuls need no on-device transposes.  The host
    permutes wq/wk columns so RoPE's even/odd pairs become the two
    partition halves, and pre-scales wq by 1/sqrt(head_dim).
  - projections run in 512-query slices into a 4-bank PSUM slab (one
    bank per head); eviction+pair-swap amortize into three big scalar
    copies per slab, then RoPE is two DVE multiplies and an add per
    head at the 2x bf16 rate.
  - the v projection reuses the same PSUM slab pool (4 chunk-chains
    per slab, one big eviction copy), so there is no pool-close
    barrier stalling the PE between the q/k and v passes.
  - x^T stays resident in SBUF for the whole projection phase.
  - scores computed as s^T (keys x q) per 256-query group; softmax
    skips the max-subtraction (scores are O(1) by construction); row
    sums via a ones-vector matmul (the PE is the cheapest engine for a
    partition-axis reduction); normalization folded into the PSUM
    eviction on the DVE.  o and l share one PSUM bank; the l chain is
    deferred until the o group closes (two groups open in one bank
    corrupt each other).
  - fully-masked key blocks are skipped (host inspects the mask);
    deduplicated exp(mask) tiles multiply pt only where a block is
    partially masked.
  - the output projection is interleaved into the attention loop (one
    PSUM bank per 512-col group, popped after every score quad), so
    its matmuls fill the PE during softmax dependency stalls instead
    of forming a serial tail, and the tensor engine never idles long
    enough to lose its p-state.
"""

import sys
from contextlib import ExitStack

if "/opt/trn_rl_repo" not in sys.path:
    sys.path.insert(0, "/opt/trn_rl_repo")

import numpy as np

import concourse.bacc as bacc
import concourse.mybir as mybir
import concourse.tile as tile
from concourse.tile import add_dep_helper
from concourse.bass_utils import run_bass_kernel_spmd

B, T, D, NH, HD = 2, 2048, 2048, 16, 128
HPC = 4              # heads per core
SL = 512             # projection slice width (max moving dim)
NSL = T // SL        # 4
PAIR = 256           # queries per attention group
NPAIR = T // PAIR    # 8
NCHUNK = T // HD     # 16 key chunks of 128
BF = mybir.dt.bfloat16
F32 = mybir.dt.float32


def _mask_structure(mask):
    """Classify each (query-group, key-chunk) block of the additive mask.

    Returns (statuses, maskt): statuses[j] is a list of
    (chunk, mask_tile_index_or_minus1) for blocks that must be computed;
    maskt is the packed (128, nmask, 256) array of deduplicated
    transposed exp(mask) tiles for partially-masked blocks.
    """
    statuses = []
    tiles = {}
    tile_list = []
    for j in range(NPAIR):
        q = slice(j * PAIR, (j + 1) * PAIR)
        lst = []
        for c in range(NCHUNK):
            k = slice(c * HD, (c + 1) * HD)
            sub = mask[q, k]
            if np.all(sub <= -1e8):
                continue
            if np.all(sub == 0.0):
                lst.append((c, -1))
            else:
                key = sub.tobytes()
                mi = tiles.get(key)
                if mi is None:
                    mi = len(tile_list)
                    tiles[key] = mi
                    tile_list.append(np.ascontiguousarray(sub.T))
                lst.append((c, mi))
        assert lst, f"query group {j} has every key block masked"
        statuses.append(lst)
    nmask = max(1, len(tile_list))
    assert nmask <= 24, "too many distinct mask tiles to preload"
    maskt = np.zeros((HD, nmask, PAIR), np.float32)
    for i, t in enumerate(tile_list):
        assert np.all(t <= 64.0), "additive mask too large for exp-mask trick"
        maskt[:, i, :] = np.exp(t)
    return statuses, maskt


def _build_program(statuses, nmask):
    nc = bacc.Bacc(None, target_bir_lowering=False)

    # all inputs partition-major: leading dim = the 128 SBUF partitions
    xt_d = nc.dram_tensor("xts", [HD, NSL, NCHUNK, SL], BF, kind="ExternalInput")
    wq_d = nc.dram_tensor("wqt", [HD, NCHUNK, HPC * HD], BF, kind="ExternalInput")
    wk_d = nc.dram_tensor("wkt", [HD, NCHUNK, HPC * HD], BF, kind="ExternalInput")
    wv_d = nc.dram_tensor("wvt", [HD, NCHUNK, HPC * HD], BF, kind="ExternalInput")
    wo_d = nc.dram_tensor("wot", [HD, HPC, D], BF, kind="ExternalInput")
    cs_d = nc.dram_tensor("cs", [HD, 2, T], BF, kind="ExternalInput")
    mk_d = nc.dram_tensor("maskt", [HD, nmask, PAIR], BF, kind="ExternalInput")
    ones_d = nc.dram_tensor("ones_col", [HD, 1], BF, kind="ExternalInput")
    out_d = nc.dram_tensor("out", [T, D], BF, kind="ExternalOutput")

    EXP = mybir.ActivationFunctionType.Exp

    with tile.TileContext(nc) as tc, ExitStack() as top:
        constp = top.enter_context(tc.tile_pool(name="const", bufs=1))
        ones_sb = constp.tile([HD, 1], BF)
        nc.scalar.dma_start(ones_sb[:], ones_d[:])
        csp = top.enter_context(tc.tile_pool(name="csp", bufs=1))
        cs_sb = csp.tile([HD, 2, T], BF)

        qkp = top.enter_context(tc.tile_pool(name="qkp", bufs=1))
        # q heads at [:, h, :], k heads at [:, 4+h, :]
        qk_sb = qkp.tile([HD, 2 * HPC, T], BF)
        vap = top.enter_context(tc.tile_pool(name="vap", bufs=1))
        v_all = vap.tile([HD, NCHUNK, HPC * HD], BF)

        mkp = top.enter_context(tc.tile_pool(name="mkp", bufs=1))
        mk_sb = mkp.tile([HD, nmask, PAIR], BF)

        # ---- projection phase: q/k slabs (+RoPE), then v slabs ----
        # DMA waves: the chip HBM is shared by all 8 cores and all
        # queues round-robin, so everything issued at t=0 finishes
        # together.  Wave 1 is only what the first two slabs need;
        # everything else waits (manual dep) on the first slab's
        # eviction so the critical transfers get full bandwidth.
        vslab_evicts = []
        with ExitStack() as ph:
            xtp = ph.enter_context(tc.tile_pool(name="xtp", side="right", bufs=1))
            xt_sb = xtp.tile([HD, NSL, NCHUNK, SL], BF)
            # the q/k pass runs slices high-to-low (attention starts at
            # the last query group), so load the last slice first, split
            # in two so the first chain can start sooner; the cos/sin
            # table is not needed until the first RoPE ~14us in
            nc.sync.dma_start(xt_sb[:, NSL - 1, 0:8, :], xt_d.ap()[:, NSL - 1, 0:8, :])
            nc.sync.dma_start(xt_sb[:, NSL - 1, 8:16, :], xt_d.ap()[:, NSL - 1, 8:16, :])
            nc.sync.dma_start(cs_sb[:], cs_d[:])

            wp = ph.enter_context(tc.tile_pool(name="wp", side="right", bufs=1))
            wvp = ph.enter_context(tc.tile_pool(name="wvp", side="right", bufs=1))
            rawp = ph.enter_context(tc.tile_pool(name="rawp", side="right", bufs=2))
            swp = ph.enter_context(tc.tile_pool(name="swp", side="right", bufs=2))
            tbp = ph.enter_context(tc.tile_pool(name="tbp", side="right", bufs=2))
            pps = ph.enter_context(tc.tile_pool(name="pps", bufs=2, space="PSUM"))
            wqk_sb = wp.tile([HD, 2, NCHUNK, HPC * HD], BF)
            wv_sb = wvp.tile([HD, NCHUNK, HPC * HD], BF)
            nc.scalar.dma_start(wqk_sb[:, 0, 0:8, :], wq_d.ap()[:, 0:8, :])
            nc.scalar.dma_start(wqk_sb[:, 0, 8:16, :], wq_d.ap()[:, 8:16, :])
            nc.gpsimd.dma_start(wqk_sb[:, 1, :, :], wk_d.ap()[:])
            nc.gpsimd.dma_start(mk_sb[:], mk_d[:])

            slab0_evict = None
            for ns in reversed(range(NSL)):
                tsl = slice(ns * SL, (ns + 1) * SL)
                for wsel in range(2):
                    ps = pps.tile([HD, HPC, SL], F32, tag="ps")
                    for h in range(HPC):
                        hs = slice(h * HD, (h + 1) * HD)
                        for k in range(NCHUNK):
                            nc.tensor.matmul(
                                ps[:, h, :],
                                wqk_sb[:, wsel, k, hs],
                                xt_sb[:, ns, k, :],
                                start=(k == 0),
                                stop=(k == NCHUNK - 1),
                            )
                    # Slab eviction: raw copy + partition-half swap,
                    # three big scalar copies (the ACT engine is the
                    # only one that can cross partitions cheaply).
                    raw = rawp.tile([HD, HPC, SL], BF, tag="raw")
                    sw = swp.tile([HD, HPC, SL], BF, tag="sw")
                    ev0 = nc.scalar.copy(raw[:], ps[:])
                    nc.scalar.copy(sw[0:64, :, :], ps[64:128, :, :])
                    nc.scalar.copy(sw[64:128, :, :], ps[0:64, :, :])
                    if slab0_evict is None:
                        slab0_evict = ev0
                        # wave 2: the rest of x^T (chained so earlier-
                        # needed slices transfer first), then wv
                        prev = slab0_evict
                        for ns2 in reversed(range(NSL - 1)):
                            dma = nc.sync.dma_start(
                                xt_sb[:, ns2, :, :], xt_d.ap()[:, ns2, :, :]
                            )
                            add_dep_helper(
                                dma.ins, prev.ins, True, "dma wave order"
                            )
                            prev = dma
                        wdma = nc.gpsimd.dma_start(wv_sb[:], wv_d.ap()[:])
                        add_dep_helper(
                            wdma.ins, slab0_evict.ins, True, "dma wave order"
                        )
                    # RoPE per head on the DVE at the bf16 2x rate:
                    # dst = raw*C + sw*S with C=[cos;cos], S=[-sin;sin]
                    for h in range(HPC):
                        dst = qk_sb[:, wsel * HPC + h, tsl]
                        tb = tbp.tile([HD, SL], BF, tag="tb")
                        nc.vector.tensor_mul(dst, raw[:, h, :], cs_sb[:, 0, tsl])
                        nc.vector.tensor_mul(tb[:], sw[:, h, :], cs_sb[:, 1, tsl])
                        nc.vector.tensor_add(dst, dst, tb[:])

            # v slabs from the same PSUM pool: descending chunk blocks
            # so the attention pass (which starts at the last query
            # group) finds its first v chunks ready immediately
            for c0 in (12, 8, 4, 0):
                ps = pps.tile([HD, HPC, SL], F32, tag="ps")
                for c in range(c0 + 3, c0 - 1, -1):
                    tcs = slice((c % 4) * HD, (c % 4 + 1) * HD)
                    for k in range(NCHUNK):
                        nc.tensor.matmul(
                            ps[:, c - c0, :],
                            xt_sb[:, c // 4, k, tcs],
                            wv_sb[:, k, :],
                            start=(k == 0),
                            stop=(k == NCHUNK - 1),
                        )
                vslab_evicts.append(nc.scalar.copy(v_all[:, c0:c0 + 4, :], ps[:]))

        # ---- attention with interleaved output projection ----
        ctxp = top.enter_context(tc.tile_pool(name="ctxp", bufs=1))
        ctx_sb = ctxp.tile([HD, HPC, T], BF)
        wop = top.enter_context(tc.tile_pool(name="wop", bufs=1))
        wo_sb = wop.tile([HD, HPC, D], BF)
        with ExitStack() as ph:
            ptp = ph.enter_context(tc.tile_pool(name="ptp", side="right", bufs=2))
            lrp = ph.enter_context(tc.tile_pool(name="lrp", side="right", bufs=2))
            rbp = ph.enter_context(tc.tile_pool(name="rbp", side="right", bufs=2))
            evp = ph.enter_context(tc.tile_pool(name="evp", side="right", bufs=3))
            sps = ph.enter_context(tc.tile_pool(name="sps", bufs=2, space="PSUM"))
            ops = ph.enter_context(tc.tile_pool(name="ops", bufs=2, space="PSUM"))
            wops = ph.enter_context(tc.tile_pool(name="wops", bufs=2, space="PSUM"))

            wo_dma = nc.gpsimd.dma_start(wo_sb[:], wo_d.ap()[:])
            add_dep_helper(wo_dma.ins, vslab_evicts[0].ins, True, "dma wave order")

            def finalize(st):
                # off the tensor engine: DVE fast-recip -> GpSimd partition
                # broadcast -> DVE multiply into ctx
                lr = lrp.tile([1, PAIR], F32, tag="lr")
                nc.vector.reciprocal_approx_fast(lr[:], st["l"])
                rb_sb = rbp.tile([HD, PAIR], F32, tag="rb")
                nc.gpsimd.partition_broadcast(rb_sb[:], lr[:])
                nc.vector.tensor_mul(
                    ctx_sb[:, st["h"], st["qsl"]], st["o"], rb_sb[:]
                )

            # outproj work items: one PSUM bank = one e-slice of 512 for
            # one 128-row t-chunk, contracted over the 4 local heads
            op_queue = []

            def push_outproj(j):
                for tck in (2 * j, 2 * j + 1):
                    for es in range(4):
                        op_queue.append((tck, es))

            def emit_outproj(evict_scalar=False):
                if not op_queue:
                    return
                tck, es = op_queue.pop(0)
                tsl = slice(tck * HD, (tck + 1) * HD)
                esl = slice(es * SL, (es + 1) * SL)
                ps = wops.tile([HD, SL], F32, tag="wo")
                for h in range(HPC):
                    nc.tensor.matmul(
                        ps[:],
                        ctx_sb[:, h, tsl],
                        wo_sb[:, h, esl],
                        start=(h == 0),
                        stop=(h == HPC - 1),
                    )
                ev = evp.tile([HD, SL], BF, tag="ev")
                if evict_scalar:
                    nc.scalar.copy(ev[:], ps[:])
                else:
                    nc.vector.tensor_copy(ev[:], ps[:])
                nc.sync.dma_start(out_d[tsl, esl], ev[:])

            def emit_ol(dq):
                # deferred p@v matmuls for an exp'd quad.  o and l share
                # one PSUM bank, so the l chain only starts after the o
                # group has closed (two accumulation groups open in the
                # same bank corrupt each other).
                pi, quad, st = dq
                h = st["h"]
                for t, (c, mi) in enumerate(quad):
                    nc.tensor.matmul(
                        st["o"],
                        v_all[:, c, h * HD:(h + 1) * HD],
                        st["pt"][:, pi + t, :],
                        start=(st["oi"] == 0),
                        stop=(st["oi"] == st["n"] - 1),
                        skip_group_check=True,
                    )
                    st["oi"] += 1
                if st["oi"] < st["n"]:
                    return False
                for idx in range(st["n"]):
                    nc.tensor.matmul(
                        st["l"],
                        ones_sb[:],
                        st["pt"][:, idx, :],
                        start=(idx == 0),
                        stop=(idx == st["n"] - 1),
                        skip_group_check=True,
                    )
                return True

            pending_ol = None
            pending_fin = None
            for j in reversed(range(NPAIR)):
                qsl = slice(j * PAIR, (j + 1) * PAIR)
                chunks = list(reversed(statuses[j]))
                n = len(chunks)
                quads = [chunks[ii:ii + 4] for ii in range(0, n, 4)]
                for h in range(HPC):
                    o_l = ops.tile([HD, 2, PAIR], F32, tag="o")
                    pt = ptp.tile([HD, NCHUNK, PAIR], BF, tag="pt")
                    st = {"o": o_l[:, 0, :], "l": o_l[0:1, 1, :],
                          "pt": pt, "h": h, "qsl": qsl, "n": n,
                          "oi": 0}
                    for qi, quad in enumerate(quads):
                        w = len(quad)
                        s_ps = sps.tile([HD, 4, PAIR], F32, tag="s")
                        for t, (c, mi) in enumerate(quad):
                            nc.tensor.matmul(
                                s_ps[:, t, :],
                                qk_sb[:, HPC + h, c * HD:(c + 1) * HD],
                                qk_sb[:, h, qsl],
                                start=True,
                                stop=True,
                            )
                        nc.scalar.activation(
                            pt[:, qi * 4:qi * 4 + w, :], s_ps[:, 0:w, :], EXP
                        )
                        # multiplicative exp-mask applied to pt
                        # (exp(s+m) == exp(s)*exp(m)), off the exp chain
                        t = 0
                        while t < w:
                            c, mi = quad[t]
                            if mi < 0:
                                t += 1
                                continue
                            r = t + 1
                            while (r < w and quad[r][1] >= 0
                                   and quad[r][1] == quad[r - 1][1] + 1):
                                r += 1
                            sl = slice(qi * 4 + t, qi * 4 + r)
                            nc.vector.tensor_mul(
                                pt[:, sl, :], pt[:, sl, :],
                                mk_sb[:, mi:mi + (r - t), :],
                            )
                            t = r
                        if pending_ol is not None:
                            if emit_ol(pending_ol):
                                pending_fin = pending_ol[2]
                            pending_ol = None
                        if pending_fin is not None and pending_fin is not st:
                            fj = pending_fin["qsl"].start // PAIR
                            fh = pending_fin["h"]
                            finalize(pending_fin)
                            pending_fin = None
                            if fh == HPC - 1:
                                push_outproj(fj)
                        emit_outproj()
                        pending_ol = (qi * 4, quad, st)
            if pending_ol is not None:
                if emit_ol(pending_ol):
                    pending_fin = pending_ol[2]
            if pending_fin is not None:
                finalize(pending_fin)
                push_outproj(0)
            while op_queue:
                emit_outproj(evict_scalar=True)
    nc.compile()
    return nc


_PERM = np.concatenate(
    [np.concatenate([np.arange(0, HD, 2), np.arange(1, HD, 2)]) + h * HD
     for h in range(HPC)]
)


def _bf16(a):
    import ml_dtypes

    return np.ascontiguousarray(
        np.asarray(a, np.float32).astype(ml_dtypes.bfloat16)
    )


def _pmaj(w):
    # (D, E) weight -> partition-major (HD, NCHUNK_of_D, E)
    d, e = w.shape
    return w.reshape(d // HD, HD, e).transpose(1, 0, 2)


def prepare(x, freqs, mask, wq, wk, wv, wo):
    """Host-side sharding/prep. Returns (nc, in_maps)."""
    x = np.asarray(x, np.float32)
    freqs = np.asarray(freqs, np.float32)
    mask = np.asarray(mask, np.float32)
    wq, wk, wv, wo = (np.asarray(w, np.float32) for w in (wq, wk, wv, wo))

    statuses, maskt = _mask_structure(mask)
    nc = _build_program(statuses, maskt.shape[1])

    scale = np.float32(1.0 / np.sqrt(HD))
    cos = np.ascontiguousarray(freqs[:, :, 0].T)  # (64, T)
    sin = np.ascontiguousarray(freqs[:, :, 1].T)
    cs = np.empty((HD, 2, T), np.float32)
    cs[0:64, 0, :] = cos
    cs[64:128, 0, :] = cos
    cs[0:64, 1, :] = -sin
    cs[64:128, 1, :] = sin

    ones_col = np.ones((HD, 1), np.float32)
    # x^T in (HD, NSL, NCHUNK, SL) partition-major blocks:
    # element (p, s, k, t) = x[s*SL + t, k*HD + p]
    xts = [
        _bf16(x[b].T.reshape(NCHUNK, HD, NSL, SL).transpose(1, 2, 0, 3))
        for b in range(B)
    ]
    cs_b = _bf16(cs)
    mk_b = _bf16(maskt)
    ones_b = _bf16(ones_col)

    in_maps = []
    for core in range(8):
        b, g = core // 4, core % 4
        cols = slice(g * HPC * HD, (g + 1) * HPC * HD)
        in_maps.append({
            "xts": xts[b],
            "wqt": _bf16(_pmaj((wq.T[:, cols] * scale)[:, _PERM])),
            "wkt": _bf16(_pmaj(wk.T[:, cols][:, _PERM])),
            "wvt": _bf16(_pmaj(wv.T[:, cols])),
            "wot": _bf16(_pmaj(wo.T[cols, :])),
            "cs": cs_b,
            "maskt": mk_b,
            "ones_col": ones_b,
        })
    return nc, in_maps


def run(x, freqs, mask, wq, wk, wv, wo, **spmd_kwargs):
    nc, in_maps = prepare(x, freqs, mask, wq, wk, wv, wo)
    res = run_bass_kernel_spmd(nc, in_maps, list(range(8)), **spmd_kwargs)
    parts = [res.results[c]["out"].astype(np.float32) for c in range(8)]
    out = np.stack([
        parts[b * 4] + parts[b * 4 + 1] + parts[b * 4 + 2] + parts[b * 4 + 3]
        for b in range(B)
    ]).astype(np.float32)
    return out, res


def kernel(x, freqs, mask, wq, wk, wv, wo):
    out, _ = run(x, freqs, mask, wq, wk, wv, wo)
    return out


# revision 23
# speedup vs baseline: 1.1814x; 1.0155x over previous
"""Trainium2 Bass kernel for causal self-attention with RoPE.

Shapes: x (2, 2048, 2048), 16 heads x 128 head_dim.
Sharding: 8 cores = 2 batch x 4 head-groups (4 heads per core).
Each core computes q/k/v projections for its heads, RoPE, causal-masked
softmax attention, and a partial output projection (its head columns of
wo); the host sums the 4 partials per batch element.

All matmul operands are bf16 (PSUM accumulation stays fp32): bf16 and
fp32r stream at the same 1 cycle/row on the PE, but bf16 halves DMA
traffic and SBUF footprint, loads stationary weights at 1 cycle/row,
and unlocks the DVE 2x packed mode for the element-wise work.

Layout strategy (per core):
  - all DRAM inputs are partition-major (the host pre-transposes), so
    every DMA moves >=16KB contiguous per partition instead of 1KB
    strided runs (which are descriptor-rate-bound at ~95GB/s).
  - q,k built in transposed layout (head_dim on partitions, t free) so
    RoPE and the score matmuls need no on-device transposes.  The host
    permutes wq/wk columns so RoPE's even/odd pairs become the two
    partition halves, and pre-scales wq by 1/sqrt(head_dim).
  - projections run in 512-query slices into a 4-bank PSUM slab (one
    bank per head); eviction+pair-swap amortize into three big scalar
    copies per slab, then RoPE is two DVE multiplies and an add per
    head at the 2x bf16 rate.
  - the v projection reuses the same PSUM slab pool (4 chunk-chains
    per slab, one big eviction copy), so there is no pool-close
    barrier stalling the PE between the q/k and v passes.
  - x^T stays resident in SBUF for the whole projection phase.
  - scores computed as s^T (keys x q) per 256-query group; softmax
    skips the max-subtraction (scores are O(1) by construction); row
    sums via a ones-vector matmul (the PE is the cheapest engine for a
    partition-axis reduction); normalization folded into the PSUM
    eviction on the DVE.  o and l share one PSUM bank; the l chain is
    deferred until the o group closes (two groups open in one bank
    corrupt each other).
  - fully-masked key blocks are skipped (host inspects the mask);
    deduplicated exp(mask) tiles multiply pt only where a block is
    partially masked.
  - the output projection is interleaved into the attention loop (one
    PSUM bank per 512-col group, popped after every score quad), so
    its matmuls fill the PE during softmax dependency stalls instead
    of forming a serial tail, and the tensor engine never idles long
    enough to lose its p-state.
"""

import sys
from contextlib import ExitStack

if "/opt/trn_rl_repo" not in sys.path:
    sys.path.insert(0, "/opt/trn_rl_repo")

import numpy as np

import concourse.bacc as bacc
import concourse.mybir as mybir
import concourse.tile as tile
from concourse.tile import add_dep_helper
from concourse.bass_utils import run_bass_kernel_spmd

B, T, D, NH, HD = 2, 2048, 2048, 16, 128
HPC = 4              # heads per core
SL = 512             # projection slice width (max moving dim)
NSL = T // SL        # 4
PAIR = 256           # queries per attention group
NPAIR = T // PAIR    # 8
NCHUNK = T // HD     # 16 key chunks of 128
BF = mybir.dt.bfloat16
F32 = mybir.dt.float32


def _mask_structure(mask):
    """Classify each (query-group, key-chunk) block of the additive mask.

    Returns (statuses, maskt): statuses[j] is a list of
    (chunk, mask_tile_index_or_minus1) for blocks that must be computed;
    maskt is the packed (128, nmask, 256) array of deduplicated
    transposed exp(mask) tiles for partially-masked blocks.
    """
    statuses = []
    tiles = {}
    tile_list = []
    for j in range(NPAIR):
        q = slice(j * PAIR, (j + 1) * PAIR)
        lst = []
        for c in range(NCHUNK):
            k = slice(c * HD, (c + 1) * HD)
            sub = mask[q, k]
            if np.all(sub <= -1e8):
                continue
            if np.all(sub == 0.0):
                lst.append((c, -1))
            else:
                key = sub.tobytes()
                mi = tiles.get(key)
                if mi is None:
                    mi = len(tile_list)
                    tiles[key] = mi
                    tile_list.append(np.ascontiguousarray(sub.T))
                lst.append((c, mi))
        assert lst, f"query group {j} has every key block masked"
        statuses.append(lst)
    nmask = max(1, len(tile_list))
    assert nmask <= 24, "too many distinct mask tiles to preload"
    maskt = np.zeros((HD, nmask, PAIR), np.float32)
    for i, t in enumerate(tile_list):
        assert np.all(t <= 64.0), "additive mask too large for exp-mask trick"
        maskt[:, i, :] = np.exp(t)
    return statuses, maskt


def _build_program(statuses, nmask):
    nc = bacc.Bacc(None, target_bir_lowering=False)

    # all inputs partition-major: leading dim = the 128 SBUF partitions
    xt_d = nc.dram_tensor("xts", [HD, NSL, NCHUNK, SL], BF, kind="ExternalInput")
    wq_d = nc.dram_tensor("wqt", [HD, NCHUNK, HPC * HD], BF, kind="ExternalInput")
    wk_d = nc.dram_tensor("wkt", [HD, NCHUNK, HPC * HD], BF, kind="ExternalInput")
    wv_d = nc.dram_tensor("wvt", [HD, NCHUNK, HPC * HD], BF, kind="ExternalInput")
    wo_d = nc.dram_tensor("wot", [HD, HPC, D], BF, kind="ExternalInput")
    cs_d = nc.dram_tensor("cs", [HD, 2, T], BF, kind="ExternalInput")
    mk_d = nc.dram_tensor("maskt", [HD, nmask, PAIR], BF, kind="ExternalInput")
    ones_d = nc.dram_tensor("ones_col", [HD, 1], BF, kind="ExternalInput")
    out_d = nc.dram_tensor("out", [T, D], BF, kind="ExternalOutput")

    EXP = mybir.ActivationFunctionType.Exp

    with tile.TileContext(nc) as tc, ExitStack() as top:
        constp = top.enter_context(tc.tile_pool(name="const", bufs=1))
        ones_sb = constp.tile([HD, 1], BF)
        nc.scalar.dma_start(ones_sb[:], ones_d[:])
        csp = top.enter_context(tc.tile_pool(name="csp", bufs=1))
        cs_sb = csp.tile([HD, 2, T], BF)

        qkp = top.enter_context(tc.tile_pool(name="qkp", bufs=1))
        # q heads at [:, h, :], k heads at [:, 4+h, :]
        qk_sb = qkp.tile([HD, 2 * HPC, T], BF)
        vap = top.enter_context(tc.tile_pool(name="vap", bufs=1))
        v_all = vap.tile([HD, NCHUNK, HPC * HD], BF)

        mkp = top.enter_context(tc.tile_pool(name="mkp", bufs=1))
        mk_sb = mkp.tile([HD, nmask, PAIR], BF)
        wop = top.enter_context(tc.tile_pool(name="wop", bufs=1))
        wo_sb = wop.tile([HD, HPC, D], BF)

        # ---- projection phase: q/k slabs (+RoPE), then v slabs ----
        # DMA waves: the chip HBM is shared by all 8 cores and every
        # in-flight transfer round-robins, so anything issued together
        # finishes together.  Stage transfers as a dependency ladder so
        # the first slab's operands get the full bandwidth, the second
        # wave rides behind it, and the bulk waits for compute to start.
        with ExitStack() as ph:
            xtp = ph.enter_context(tc.tile_pool(name="xtp", side="right", bufs=1))
            xt_sb = xtp.tile([HD, NSL, NCHUNK, SL], BF)
            LS = NSL - 1
            # wave 0: first half of the last x^T slice (the q/k pass
            # runs slices high-to-low), its cos/sin slice, wq first half
            xt3a = nc.sync.dma_start(xt_sb[:, LS, 0:8, :], xt_d.ap()[:, LS, 0:8, :])
            nc.sync.dma_start(
                cs_sb[:, :, LS * SL:T], cs_d[:, :, LS * SL:T]
            )
            # wave 0b: second halves, wk, remaining cos/sin
            xt3b = nc.sync.dma_start(xt_sb[:, LS, 8:16, :], xt_d.ap()[:, LS, 8:16, :])
            add_dep_helper(xt3b.ins, xt3a.ins, True, "dma wave")
            nc.sync.dma_start(cs_sb[:, :, 0:LS * SL], cs_d[:, :, 0:LS * SL])

            wp = ph.enter_context(tc.tile_pool(name="wp", side="right", bufs=1))
            wvp = ph.enter_context(tc.tile_pool(name="wvp", side="right", bufs=1))
            rawp = ph.enter_context(tc.tile_pool(name="rawp", side="right", bufs=1))
            swp = ph.enter_context(tc.tile_pool(name="swp", side="right", bufs=1))
            tbp = ph.enter_context(tc.tile_pool(name="tbp", side="right", bufs=2))
            pps = ph.enter_context(tc.tile_pool(name="pps", bufs=2, space="PSUM"))
            wqk_sb = wp.tile([HD, 2, NCHUNK, HPC * HD], BF)
            wv_sb = wvp.tile([HD, NCHUNK, HPC * HD], BF)
            wqa = nc.scalar.dma_start(wqk_sb[:, 0, 0:8, :], wq_d.ap()[:, 0:8, :])
            wqb = nc.scalar.dma_start(wqk_sb[:, 0, 8:16, :], wq_d.ap()[:, 8:16, :])
            add_dep_helper(wqb.ins, wqa.ins, True, "dma wave")
            nc.gpsimd.dma_start(mk_sb[:], mk_d[:])
            wk = nc.gpsimd.dma_start(wqk_sb[:, 1, :, :], wk_d.ap()[:])
            add_dep_helper(wk.ins, xt3a.ins, True, "dma wave")

            slab0_evict = None
            for ns in reversed(range(NSL)):
                tsl = slice(ns * SL, (ns + 1) * SL)
                for wsel in range(2):
                    ps = pps.tile([HD, HPC, SL], F32, tag="ps")
                    if slab0_evict is None:
                        # first slab: run all heads over chunks 0-7,
                        # then 8-15, so compute starts as soon as the
                        # wave-0 halves land (4 open groups in 4
                        # different PSUM banks is fine)
                        for klo, khi in ((0, 8), (8, 16)):
                            for h in range(HPC):
                                hs = slice(h * HD, (h + 1) * HD)
                                for k in range(klo, khi):
                                    nc.tensor.matmul(
                                        ps[:, h, :],
                                        wqk_sb[:, wsel, k, hs],
                                        xt_sb[:, ns, k, :],
                                        start=(k == 0),
                                        stop=(k == NCHUNK - 1),
                                        skip_group_check=True,
                                    )
                    else:
                        for h in range(HPC):
                            hs = slice(h * HD, (h + 1) * HD)
                            for k in range(NCHUNK):
                                nc.tensor.matmul(
                                    ps[:, h, :],
                                    wqk_sb[:, wsel, k, hs],
                                    xt_sb[:, ns, k, :],
                                    start=(k == 0),
                                    stop=(k == NCHUNK - 1),
                                )
                    # Slab eviction: raw copy + partition-half swap,
                    # three big scalar copies (the ACT engine is the
                    # only one that can cross partitions cheaply).
                    raw = rawp.tile([HD, HPC, SL], BF, tag="raw")
                    sw = swp.tile([HD, HPC, SL], BF, tag="sw")
                    ev0 = nc.scalar.copy(raw[:], ps[:])
                    nc.scalar.copy(sw[0:64, :, :], ps[64:128, :, :])
                    nc.scalar.copy(sw[64:128, :, :], ps[0:64, :, :])
                    if slab0_evict is None:
                        slab0_evict = ev0
                        # bulk wave: the rest of x^T (chained so
                        # earlier-needed slices transfer first), then
                        # wv and wo riding behind
                        prev = slab0_evict
                        for ns2 in reversed(range(NSL - 1)):
                            dma = nc.sync.dma_start(
                                xt_sb[:, ns2, :, :], xt_d.ap()[:, ns2, :, :]
                            )
                            add_dep_helper(dma.ins, prev.ins, True, "dma wave")
                            prev = dma
                        wvdma = nc.gpsimd.dma_start(wv_sb[:], wv_d.ap()[:])
                        add_dep_helper(
                            wvdma.ins, slab0_evict.ins, True, "dma wave"
                        )
                        wodma = nc.gpsimd.dma_start(wo_sb[:], wo_d.ap()[:])
                        add_dep_helper(wodma.ins, wvdma.ins, True, "dma wave")
                    # RoPE per head on the DVE at the bf16 2x rate:
                    # dst = raw*C + sw*S with C=[cos;cos], S=[-sin;sin]
                    for h in range(HPC):
                        dst = qk_sb[:, wsel * HPC + h, tsl]
                        tb = tbp.tile([HD, SL], BF, tag="tb")
                        nc.vector.tensor_mul(dst, raw[:, h, :], cs_sb[:, 0, tsl])
                        nc.vector.tensor_mul(tb[:], sw[:, h, :], cs_sb[:, 1, tsl])
                        nc.vector.tensor_add(dst, dst, tb[:])

            # v slabs from the same PSUM pool: descending chunk blocks
            # so the attention pass (which starts at the last query
            # group) finds its first v chunks ready immediately
            for c0 in (12, 8, 4, 0):
                ps = pps.tile([HD, HPC, SL], F32, tag="ps")
                for c in range(c0 + 3, c0 - 1, -1):
                    tcs = slice((c % 4) * HD, (c % 4 + 1) * HD)
                    for k in range(NCHUNK):
                        nc.tensor.matmul(
                            ps[:, c - c0, :],
                            xt_sb[:, c // 4, k, tcs],
                            wv_sb[:, k, :],
                            start=(k == 0),
                            stop=(k == NCHUNK - 1),
                        )
                nc.scalar.copy(v_all[:, c0:c0 + 4, :], ps[:])

        # ---- attention with interleaved output projection ----
        ctxp = top.enter_context(tc.tile_pool(name="ctxp", bufs=1))
        ctx_sb = ctxp.tile([HD, HPC, T], BF)
        with ExitStack() as ph:
            ptp = ph.enter_context(tc.tile_pool(name="ptp", side="right", bufs=2))
            lrp = ph.enter_context(tc.tile_pool(name="lrp", side="right", bufs=2))
            rbp = ph.enter_context(tc.tile_pool(name="rbp", side="right", bufs=2))
            evp = ph.enter_context(tc.tile_pool(name="evp", side="right", bufs=3))
            sps = ph.enter_context(tc.tile_pool(name="sps", bufs=2, space="PSUM"))
            ops = ph.enter_context(tc.tile_pool(name="ops", bufs=2, space="PSUM"))
            wops = ph.enter_context(tc.tile_pool(name="wops", bufs=2, space="PSUM"))

            def finalize(st):
                # off the tensor engine: DVE fast-recip -> GpSimd partition
                # broadcast -> DVE multiply into ctx
                lr = lrp.tile([1, PAIR], F32, tag="lr")
                nc.vector.reciprocal_approx_fast(lr[:], st["l"])
                rb_sb = rbp.tile([HD, PAIR], F32, tag="rb")
                nc.gpsimd.partition_broadcast(rb_sb[:], lr[:])
                nc.vector.tensor_mul(
                    ctx_sb[:, st["h"], st["qsl"]], st["o"], rb_sb[:]
                )

            # outproj work items: one PSUM bank = one e-slice of 512 for
            # one 128-row t-chunk, contracted over the 4 local heads
            op_queue = []

            def push_outproj(j):
                for tck in (2 * j, 2 * j + 1):
                    for es in range(4):
                        op_queue.append((tck, es))

            def emit_outproj(evict_scalar=False):
                if not op_queue:
                    return
                tck, es = op_queue.pop(0)
                tsl = slice(tck * HD, (tck + 1) * HD)
                esl = slice(es * SL, (es + 1) * SL)
                ps = wops.tile([HD, SL], F32, tag="wo")
                for h in range(HPC):
                    nc.tensor.matmul(
                        ps[:],
                        ctx_sb[:, h, tsl],
                        wo_sb[:, h, esl],
                        start=(h == 0),
                        stop=(h == HPC - 1),
                    )
                ev = evp.tile([HD, SL], BF, tag="ev")
                if evict_scalar:
                    nc.scalar.copy(ev[:], ps[:])
                else:
                    nc.vector.tensor_copy(ev[:], ps[:])
                nc.sync.dma_start(out_d[tsl, esl], ev[:])

            def emit_ol(dq):
                # deferred p@v matmuls for an exp'd quad.  o and l share
                # one PSUM bank, so the l chain only starts after the o
                # group has closed (two accumulation groups open in the
                # same bank corrupt each other).
                pi, quad, st = dq
                h = st["h"]
                for t, (c, mi) in enumerate(quad):
                    nc.tensor.matmul(
                        st["o"],
                        v_all[:, c, h * HD:(h + 1) * HD],
                        st["pt"][:, pi + t, :],
                        start=(st["oi"] == 0),
                        stop=(st["oi"] == st["n"] - 1),
                        skip_group_check=True,
                    )
                    st["oi"] += 1
                if st["oi"] < st["n"]:
                    return False
                for idx in range(st["n"]):
                    nc.tensor.matmul(
                        st["l"],
                        ones_sb[:],
                        st["pt"][:, idx, :],
                        start=(idx == 0),
                        stop=(idx == st["n"] - 1),
                        skip_group_check=True,
                    )
                return True

            pending_ol = None
            pending_fin = None
            for j in reversed(range(NPAIR)):
                qsl = slice(j * PAIR, (j + 1) * PAIR)
                chunks = list(reversed(statuses[j]))
                n = len(chunks)
                quads = [chunks[ii:ii + 4] for ii in range(0, n, 4)]
                for h in range(HPC):
                    o_l = ops.tile([HD, 2, PAIR], F32, tag="o")
                    pt = ptp.tile([HD, NCHUNK, PAIR], BF, tag="pt")
                    st = {"o": o_l[:, 0, :], "l": o_l[0:1, 1, :],
                          "pt": pt, "h": h, "qsl": qsl, "n": n,
                          "oi": 0}
                    for qi, quad in enumerate(quads):
                        w = len(quad)
                        s_ps = sps.tile([HD, 4, PAIR], F32, tag="s")
                        for t, (c, mi) in enumerate(quad):
                            nc.tensor.matmul(
                                s_ps[:, t, :],
                                qk_sb[:, HPC + h, c * HD:(c + 1) * HD],
                                qk_sb[:, h, qsl],
                                start=True,
                                stop=True,
                            )
                        nc.scalar.activation(
                            pt[:, qi * 4:qi * 4 + w, :], s_ps[:, 0:w, :], EXP
                        )
                        # multiplicative exp-mask applied to pt
                        # (exp(s+m) == exp(s)*exp(m)), off the exp chain
                        t = 0
                        while t < w:
                            c, mi = quad[t]
                            if mi < 0:
                                t += 1
                                continue
                            r = t + 1
                            while (r < w and quad[r][1] >= 0
                                   and quad[r][1] == quad[r - 1][1] + 1):
                                r += 1
                            sl = slice(qi * 4 + t, qi * 4 + r)
                            nc.vector.tensor_mul(
                                pt[:, sl, :], pt[:, sl, :],
                                mk_sb[:, mi:mi + (r - t), :],
                            )
                            t = r
                        if pending_ol is not None:
                            if emit_ol(pending_ol):
                                pending_fin = pending_ol[2]
                            pending_ol = None
                        if pending_fin is not None and pending_fin is not st:
                            fj = pending_fin["qsl"].start // PAIR
                            fh = pending_fin["h"]
                            finalize(pending_fin)
                            pending_fin = None
                            if fh == HPC - 1:
                                push_outproj(fj)
                        emit_outproj()
                        pending_ol = (qi * 4, quad, st)
            if pending_ol is not None:
                if emit_ol(pending_ol):
                    pending_fin = pending_ol[2]
            if pending_fin is not None:
                finalize(pending_fin)
                push_outproj(0)
            while op_queue:
                emit_outproj(evict_scalar=True)
    nc.compile()
    return nc


_PERM = np.concatenate(
    [np.concatenate([np.arange(0, HD, 2), np.arange(1, HD, 2)]) + h * HD
     for h in range(HPC)]
)


def _bf16(a):
    import ml_dtypes

    return np.ascontiguousarray(
        np.asarray(a, np.float32).astype(ml_dtypes.bfloat16)
    )


def _pmaj(w):
    # (D, E) weight -> partition-major (HD, NCHUNK_of_D, E)
    d, e = w.shape
    return w.reshape(d // HD, HD, e).transpose(1, 0, 2)


def prepare(x, freqs, mask, wq, wk, wv, wo):
    """Host-side sharding/prep. Returns (nc, in_maps)."""
    x = np.asarray(x, np.float32)
    freqs = np.asarray(freqs, np.float32)
    mask = np.asarray(mask, np.float32)
    wq, wk, wv, wo = (np.asarray(w, np.float32) for w in (wq, wk, wv, wo))

    statuses, maskt = _mask_structure(mask)
    nc = _build_program(statuses, maskt.shape[1])

    scale = np.float32(1.0 / np.sqrt(HD))
    cos = np.ascontiguousarray(freqs[:, :, 0].T)  # (64, T)
    sin = np.ascontiguousarray(freqs[:, :, 1].T)
    cs = np.empty((HD, 2, T), np.float32)
    cs[0:64, 0, :] = cos
    cs[64:128, 0, :] = cos
    cs[0:64, 1, :] = -sin
    cs[64:128, 1, :] = sin

    ones_col = np.ones((HD, 1), np.float32)
    # x^T in (HD, NSL, NCHUNK, SL) partition-major blocks:
    # element (p, s, k, t) = x[s*SL + t, k*HD + p]
    xts = [
        _bf16(x[b].T.reshape(NCHUNK, HD, NSL, SL).transpose(1, 2, 0, 3))
        for b in range(B)
    ]
    cs_b = _bf16(cs)
    mk_b = _bf16(maskt)
    ones_b = _bf16(ones_col)

    in_maps = []
    for core in range(8):
        b, g = core // 4, core % 4
        cols = slice(g * HPC * HD, (g + 1) * HPC * HD)
        in_maps.append({
            "xts": xts[b],
            "wqt": _bf16(_pmaj((wq.T[:, cols] * scale)[:, _PERM])),
            "wkt": _bf16(_pmaj(wk.T[:, cols][:, _PERM])),
            "wvt": _bf16(_pmaj(wv.T[:, cols])),
            "wot": _bf16(_pmaj(wo.T[cols, :])),
            "cs": cs_b,
            "maskt": mk_b,
            "ones_col": ones_b,
        })
    return nc, in_maps


def run(x, freqs, mask, wq, wk, wv, wo, **spmd_kwargs):
    nc, in_maps = prepare(x, freqs, mask, wq, wk, wv, wo)
    res = run_bass_kernel_spmd(nc, in_maps, list(range(8)), **spmd_kwargs)
    parts = [res.results[c]["out"].astype(np.float32) for c in range(8)]
    out = np.stack([
        parts[b * 4] + parts[b * 4 + 1] + parts[b * 4 + 2] + parts[b * 4 + 3]
        for b in range(B)
    ]).astype(np.float32)
    return out, res


def kernel(x, freqs, mask, wq, wk, wv, wo):
    out, _ = run(x, freqs, mask, wq, wk, wv, wo)
    return out


# revision 25
# speedup vs baseline: 1.2282x; 1.0396x over previous
"""Trainium2 Bass kernel for causal self-attention with RoPE.

Shapes: x (2, 2048, 2048), 16 heads x 128 head_dim.
Sharding: 8 cores = 2 batch x 4 head-groups (4 heads per core).
Each core computes q/k/v projections for its heads, RoPE, causal-masked
softmax attention, and a partial output projection (its head columns of
wo); the host sums the 4 partials per batch element.

All matmul operands are bf16 (PSUM accumulation stays fp32): bf16 and
fp32r stream at the same 1 cycle/row on the PE, but bf16 halves DMA
traffic and SBUF footprint, loads stationary weights at 1 cycle/row,
and unlocks the DVE 2x packed mode for the element-wise work.

Layout strategy (per core):
  - all DRAM inputs are partition-major (the host pre-transposes), so
    every DMA moves >=16KB contiguous per partition instead of 1KB
    strided runs (which are descriptor-rate-bound at ~95GB/s).
  - q,k built in transposed layout (head_dim on partitions, t free) so
    RoPE and the score matmuls need no on-device transposes.  The host
    permutes wq/wk columns so RoPE's even/odd pairs become the two
    partition halves, and pre-scales wq by 1/sqrt(head_dim).
  - projections run in 512-query slices into a 4-bank PSUM slab (one
    bank per head); eviction+pair-swap amortize into three big scalar
    copies per slab, then RoPE is two DVE multiplies and an add per
    head at the 2x bf16 rate.
  - the v projection reuses the same PSUM slab pool (4 chunk-chains
    per slab, one big eviction copy), so there is no pool-close
    barrier stalling the PE between the q/k and v passes.
  - x^T stays resident in SBUF for the whole projection phase.
  - scores computed as s^T (keys x q) per 256-query group; softmax
    skips the max-subtraction (scores are O(1) by construction); row
    sums via a ones-vector matmul (the PE is the cheapest engine for a
    partition-axis reduction); normalization folded into the PSUM
    eviction on the DVE.  o and l share one PSUM bank; the l chain is
    deferred until the o group closes (two groups open in one bank
    corrupt each other).
  - fully-masked key blocks are skipped (host inspects the mask);
    deduplicated exp(mask) tiles multiply pt only where a block is
    partially masked.
  - the output projection is interleaved into the attention loop (one
    PSUM bank per 512-col group, popped after every score quad), so
    its matmuls fill the PE during softmax dependency stalls instead
    of forming a serial tail, and the tensor engine never idles long
    enough to lose its p-state.
"""

import sys
from contextlib import ExitStack

if "/opt/trn_rl_repo" not in sys.path:
    sys.path.insert(0, "/opt/trn_rl_repo")

import numpy as np

import concourse.bacc as bacc
import concourse.mybir as mybir
import concourse.tile as tile
from concourse.tile import add_dep_helper
from concourse.bass_utils import run_bass_kernel_spmd

B, T, D, NH, HD = 2, 2048, 2048, 16, 128
HPC = 4              # heads per core
SL = 512             # projection slice width (max moving dim)
NSL = T // SL        # 4
PAIR = 256           # queries per attention group
NPAIR = T // PAIR    # 8
NCHUNK = T // HD     # 16 key chunks of 128
BF = mybir.dt.bfloat16
F32 = mybir.dt.float32


def _mask_structure(mask):
    """Classify each (query-group, key-chunk) block of the additive mask.

    Returns (statuses, maskt): statuses[j] is a list of
    (chunk, mask_tile_index_or_minus1) for blocks that must be computed;
    maskt is the packed (128, nmask, 256) array of deduplicated
    transposed exp(mask) tiles for partially-masked blocks.
    """
    statuses = []
    tiles = {}
    tile_list = []
    for j in range(NPAIR):
        q = slice(j * PAIR, (j + 1) * PAIR)
        lst = []
        for c in range(NCHUNK):
            k = slice(c * HD, (c + 1) * HD)
            sub = mask[q, k]
            if np.all(sub <= -1e8):
                continue
            if np.all(sub == 0.0):
                lst.append((c, -1))
            else:
                key = sub.tobytes()
                mi = tiles.get(key)
                if mi is None:
                    mi = len(tile_list)
                    tiles[key] = mi
                    tile_list.append(np.ascontiguousarray(sub.T))
                lst.append((c, mi))
        assert lst, f"query group {j} has every key block masked"
        statuses.append(lst)
    nmask = max(1, len(tile_list))
    assert nmask <= 24, "too many distinct mask tiles to preload"
    maskt = np.zeros((HD, nmask, PAIR), np.float32)
    for i, t in enumerate(tile_list):
        assert np.all(t <= 64.0), "additive mask too large for exp-mask trick"
        maskt[:, i, :] = np.exp(t)
    return statuses, maskt


def _build_program(statuses, nmask):
    nc = bacc.Bacc(None, target_bir_lowering=False)

    # all inputs partition-major: leading dim = the 128 SBUF partitions
    xt_d = nc.dram_tensor("xts", [HD, NSL, NCHUNK, SL], BF, kind="ExternalInput")
    wq_d = nc.dram_tensor("wqt", [HD, NCHUNK, HPC * HD], BF, kind="ExternalInput")
    wk_d = nc.dram_tensor("wkt", [HD, NCHUNK, HPC * HD], BF, kind="ExternalInput")
    wv_d = nc.dram_tensor("wvt", [HD, NCHUNK, HPC * HD], BF, kind="ExternalInput")
    wo_d = nc.dram_tensor("wot", [HD, HPC, D], BF, kind="ExternalInput")
    cs_d = nc.dram_tensor("cs", [HD, 2, T], BF, kind="ExternalInput")
    mk_d = nc.dram_tensor("maskt", [HD, nmask, PAIR], BF, kind="ExternalInput")
    ones_d = nc.dram_tensor("ones_col", [HD, 1], BF, kind="ExternalInput")
    out_d = nc.dram_tensor("out", [T, D], BF, kind="ExternalOutput")

    EXP = mybir.ActivationFunctionType.Exp

    with tile.TileContext(nc) as tc, ExitStack() as top:
        constp = top.enter_context(tc.tile_pool(name="const", bufs=1))
        ones_sb = constp.tile([HD, 1], BF)
        nc.scalar.dma_start(ones_sb[:], ones_d[:])
        csp = top.enter_context(tc.tile_pool(name="csp", bufs=1))
        cs_sb = csp.tile([HD, 2, T], BF)

        qkp = top.enter_context(tc.tile_pool(name="qkp", bufs=1))
        # q heads at [:, h, :], k heads at [:, 4+h, :]
        qk_sb = qkp.tile([HD, 2 * HPC, T], BF)
        vap = top.enter_context(tc.tile_pool(name="vap", bufs=1))
        v_all = vap.tile([HD, NCHUNK, HPC * HD], BF)

        mkp = top.enter_context(tc.tile_pool(name="mkp", bufs=1))
        mk_sb = mkp.tile([HD, nmask, PAIR], BF)
        wop = top.enter_context(tc.tile_pool(name="wop", bufs=1))
        wo_sb = wop.tile([HD, HPC, D], BF)

        # ---- projection phase: q/k slabs (+RoPE), then v slabs ----
        # DMA waves: the chip HBM is shared by all 8 cores and every
        # in-flight transfer round-robins, so anything issued together
        # finishes together.  Stage transfers as a dependency ladder so
        # the first slab's operands get the full bandwidth, the second
        # wave rides behind it, and the bulk waits for compute to start.
        with ExitStack() as ph:
            xtp = ph.enter_context(tc.tile_pool(name="xtp", side="right", bufs=1))
            xt_sb = xtp.tile([HD, NSL, NCHUNK, SL], BF)
            LS = NSL - 1
            # wave 0: first half of the last x^T slice (the q/k pass
            # runs slices high-to-low), its cos/sin slice, wq first half
            xt3a = nc.sync.dma_start(xt_sb[:, LS, 0:8, :], xt_d.ap()[:, LS, 0:8, :])
            nc.sync.dma_start(
                cs_sb[:, :, LS * SL:T], cs_d[:, :, LS * SL:T]
            )
            # wave 0b: second halves, wk, remaining cos/sin
            xt3b = nc.sync.dma_start(xt_sb[:, LS, 8:16, :], xt_d.ap()[:, LS, 8:16, :])
            add_dep_helper(xt3b.ins, xt3a.ins, True, "dma wave")
            csr = nc.sync.dma_start(cs_sb[:, :, 0:LS * SL], cs_d[:, :, 0:LS * SL])
            add_dep_helper(csr.ins, xt3b.ins, True, "dma wave")

            wp = ph.enter_context(tc.tile_pool(name="wp", side="right", bufs=1))
            wvp = ph.enter_context(tc.tile_pool(name="wvp", side="right", bufs=1))
            rawp = ph.enter_context(tc.tile_pool(name="rawp", side="right", bufs=1))
            swp = ph.enter_context(tc.tile_pool(name="swp", side="right", bufs=1))
            tbp = ph.enter_context(tc.tile_pool(name="tbp", side="right", bufs=2))
            pps = ph.enter_context(tc.tile_pool(name="pps", bufs=2, space="PSUM"))
            wqk_sb = wp.tile([HD, 2, NCHUNK, HPC * HD], BF)
            wv_sb = wvp.tile([HD, NCHUNK, HPC * HD], BF)
            wqa = nc.scalar.dma_start(wqk_sb[:, 0, 0:8, :], wq_d.ap()[:, 0:8, :])
            wqb = nc.scalar.dma_start(wqk_sb[:, 0, 8:16, :], wq_d.ap()[:, 8:16, :])
            add_dep_helper(wqb.ins, wqa.ins, True, "dma wave")
            nc.gpsimd.dma_start(mk_sb[:], mk_d[:])
            wk = nc.gpsimd.dma_start(wqk_sb[:, 1, :, :], wk_d.ap()[:])
            add_dep_helper(wk.ins, xt3b.ins, True, "dma wave")
            # warm the gpsimd custom-op library now: the first
            # partition_broadcast otherwise pays a ~7us LOAD_LIB stall
            # in the middle of the first attention finalize
            warm = constp.tile([HD, 1], BF)
            nc.gpsimd.partition_broadcast(warm[:], ones_sb[0:1, :])

            slab0_evict = None
            for ns in reversed(range(NSL)):
                tsl = slice(ns * SL, (ns + 1) * SL)
                for wsel in range(2):
                    ps = pps.tile([HD, HPC, SL], F32, tag="ps")
                    if slab0_evict is None:
                        # first slab: run all heads over chunks 0-7,
                        # then 8-15, so compute starts as soon as the
                        # wave-0 halves land (4 open groups in 4
                        # different PSUM banks is fine)
                        for klo, khi in ((0, 8), (8, 16)):
                            for h in range(HPC):
                                hs = slice(h * HD, (h + 1) * HD)
                                for k in range(klo, khi):
                                    nc.tensor.matmul(
                                        ps[:, h, :],
                                        wqk_sb[:, wsel, k, hs],
                                        xt_sb[:, ns, k, :],
                                        start=(k == 0),
                                        stop=(k == NCHUNK - 1),
                                        skip_group_check=True,
                                    )
                    else:
                        for h in range(HPC):
                            hs = slice(h * HD, (h + 1) * HD)
                            for k in range(NCHUNK):
                                nc.tensor.matmul(
                                    ps[:, h, :],
                                    wqk_sb[:, wsel, k, hs],
                                    xt_sb[:, ns, k, :],
                                    start=(k == 0),
                                    stop=(k == NCHUNK - 1),
                                )
                    # Slab eviction: raw copy + partition-half swap,
                    # three big scalar copies (the ACT engine is the
                    # only one that can cross partitions cheaply).
                    raw = rawp.tile([HD, HPC, SL], BF, tag="raw")
                    sw = swp.tile([HD, HPC, SL], BF, tag="sw")
                    ev0 = nc.scalar.copy(raw[:], ps[:])
                    nc.scalar.copy(sw[0:64, :, :], ps[64:128, :, :])
                    nc.scalar.copy(sw[64:128, :, :], ps[0:64, :, :])
                    if slab0_evict is None:
                        slab0_evict = ev0
                        # bulk wave: the rest of x^T (chained so
                        # earlier-needed slices transfer first), then
                        # wv and wo riding behind
                        prev = slab0_evict
                        for ns2 in reversed(range(NSL - 1)):
                            dma = nc.sync.dma_start(
                                xt_sb[:, ns2, :, :], xt_d.ap()[:, ns2, :, :]
                            )
                            add_dep_helper(dma.ins, prev.ins, True, "dma wave")
                            prev = dma
                        wvdma = nc.gpsimd.dma_start(wv_sb[:], wv_d.ap()[:])
                        add_dep_helper(
                            wvdma.ins, slab0_evict.ins, True, "dma wave"
                        )
                        wodma = nc.gpsimd.dma_start(wo_sb[:], wo_d.ap()[:])
                        add_dep_helper(wodma.ins, wvdma.ins, True, "dma wave")
                    # RoPE per head on the DVE at the bf16 2x rate:
                    # dst = raw*C + sw*S with C=[cos;cos], S=[-sin;sin]
                    for h in range(HPC):
                        dst = qk_sb[:, wsel * HPC + h, tsl]
                        tb = tbp.tile([HD, SL], BF, tag="tb")
                        nc.vector.tensor_mul(dst, raw[:, h, :], cs_sb[:, 0, tsl])
                        nc.vector.tensor_mul(tb[:], sw[:, h, :], cs_sb[:, 1, tsl])
                        nc.vector.tensor_add(dst, dst, tb[:])

            # v slabs from the same PSUM pool: descending chunk blocks
            # so the attention pass (which starts at the last query
            # group) finds its first v chunks ready immediately
            for c0 in (12, 8, 4, 0):
                ps = pps.tile([HD, HPC, SL], F32, tag="ps")
                for c in range(c0 + 3, c0 - 1, -1):
                    tcs = slice((c % 4) * HD, (c % 4 + 1) * HD)
                    for k in range(NCHUNK):
                        nc.tensor.matmul(
                            ps[:, c - c0, :],
                            xt_sb[:, c // 4, k, tcs],
                            wv_sb[:, k, :],
                            start=(k == 0),
                            stop=(k == NCHUNK - 1),
                        )
                nc.scalar.copy(v_all[:, c0:c0 + 4, :], ps[:])

        # ---- attention with interleaved output projection ----
        ctxp = top.enter_context(tc.tile_pool(name="ctxp", bufs=1))
        ctx_sb = ctxp.tile([HD, HPC, T], BF)
        with ExitStack() as ph:
            ptp = ph.enter_context(tc.tile_pool(name="ptp", side="right", bufs=2))
            lrp = ph.enter_context(tc.tile_pool(name="lrp", side="right", bufs=2))
            rbp = ph.enter_context(tc.tile_pool(name="rbp", side="right", bufs=2))
            evp = ph.enter_context(tc.tile_pool(name="evp", side="right", bufs=3))
            sps = ph.enter_context(tc.tile_pool(name="sps", bufs=2, space="PSUM"))
            ops = ph.enter_context(tc.tile_pool(name="ops", bufs=2, space="PSUM"))
            wops = ph.enter_context(tc.tile_pool(name="wops", bufs=2, space="PSUM"))

            def finalize(st):
                # off the tensor engine: DVE fast-recip -> GpSimd partition
                # broadcast -> DVE multiply into ctx
                lr = lrp.tile([1, PAIR], F32, tag="lr")
                nc.vector.reciprocal_approx_fast(lr[:], st["l"])
                rb_sb = rbp.tile([HD, PAIR], F32, tag="rb")
                nc.gpsimd.partition_broadcast(rb_sb[:], lr[:])
                nc.vector.tensor_mul(
                    ctx_sb[:, st["h"], st["qsl"]], st["o"], rb_sb[:]
                )

            # outproj work items: one PSUM bank = one e-slice of 512 for
            # one 128-row t-chunk, contracted over the 4 local heads
            op_queue = []

            def push_outproj(j):
                for tck in (2 * j, 2 * j + 1):
                    for es in range(4):
                        op_queue.append((tck, es))

            def emit_outproj(evict_scalar=False):
                if not op_queue:
                    return
                tck, es = op_queue.pop(0)
                tsl = slice(tck * HD, (tck + 1) * HD)
                esl = slice(es * SL, (es + 1) * SL)
                ps = wops.tile([HD, SL], F32, tag="wo")
                for h in range(HPC):
                    nc.tensor.matmul(
                        ps[:],
                        ctx_sb[:, h, tsl],
                        wo_sb[:, h, esl],
                        start=(h == 0),
                        stop=(h == HPC - 1),
                    )
                ev = evp.tile([HD, SL], BF, tag="ev")
                if evict_scalar:
                    nc.scalar.copy(ev[:], ps[:])
                else:
                    nc.vector.tensor_copy(ev[:], ps[:])
                nc.sync.dma_start(out_d[tsl, esl], ev[:])

            def emit_ol(dq):
                # deferred p@v matmuls for an exp'd quad.  o and l share
                # one PSUM bank, so the l chain only starts after the o
                # group has closed (two accumulation groups open in the
                # same bank corrupt each other).
                pi, quad, st = dq
                h = st["h"]
                for t, (c, mi) in enumerate(quad):
                    nc.tensor.matmul(
                        st["o"],
                        v_all[:, c, h * HD:(h + 1) * HD],
                        st["pt"][:, pi + t, :],
                        start=(st["oi"] == 0),
                        stop=(st["oi"] == st["n"] - 1),
                        skip_group_check=True,
                    )
                    st["oi"] += 1
                if st["oi"] < st["n"]:
                    return False
                for idx in range(st["n"]):
                    nc.tensor.matmul(
                        st["l"],
                        ones_sb[:],
                        st["pt"][:, idx, :],
                        start=(idx == 0),
                        stop=(idx == st["n"] - 1),
                        skip_group_check=True,
                    )
                return True

            pending_ol = None
            pending_fin = None
            for j in reversed(range(NPAIR)):
                qsl = slice(j * PAIR, (j + 1) * PAIR)
                chunks = list(reversed(statuses[j]))
                n = len(chunks)
                quads = [chunks[ii:ii + 4] for ii in range(0, n, 4)]
                for h in range(HPC):
                    o_l = ops.tile([HD, 2, PAIR], F32, tag="o")
                    pt = ptp.tile([HD, NCHUNK, PAIR], BF, tag="pt")
                    st = {"o": o_l[:, 0, :], "l": o_l[0:1, 1, :],
                          "pt": pt, "h": h, "qsl": qsl, "n": n,
                          "oi": 0}
                    for qi, quad in enumerate(quads):
                        w = len(quad)
                        s_ps = sps.tile([HD, 4, PAIR], F32, tag="s")
                        for t, (c, mi) in enumerate(quad):
                            nc.tensor.matmul(
                                s_ps[:, t, :],
                                qk_sb[:, HPC + h, c * HD:(c + 1) * HD],
                                qk_sb[:, h, qsl],
                                start=True,
                                stop=True,
                            )
                        nc.scalar.activation(
                            pt[:, qi * 4:qi * 4 + w, :], s_ps[:, 0:w, :], EXP
                        )
                        # multiplicative exp-mask applied to pt
                        # (exp(s+m) == exp(s)*exp(m)), off the exp chain
                        t = 0
                        while t < w:
                            c, mi = quad[t]
                            if mi < 0:
                                t += 1
                                continue
                            r = t + 1
                            while (r < w and quad[r][1] >= 0
                                   and quad[r][1] == quad[r - 1][1] + 1):
                                r += 1
                            sl = slice(qi * 4 + t, qi * 4 + r)
                            nc.vector.tensor_mul(
                                pt[:, sl, :], pt[:, sl, :],
                                mk_sb[:, mi:mi + (r - t), :],
                            )
                            t = r
                        if pending_ol is not None:
                            if emit_ol(pending_ol):
                                pending_fin = pending_ol[2]
                            pending_ol = None
                        if pending_fin is not None and pending_fin is not st:
                            fj = pending_fin["qsl"].start // PAIR
                            fh = pending_fin["h"]
                            finalize(pending_fin)
                            pending_fin = None
                            if fh == HPC - 1:
                                push_outproj(fj)
                        emit_outproj()
                        pending_ol = (qi * 4, quad, st)
            if pending_ol is not None:
                if emit_ol(pending_ol):
                    pending_fin = pending_ol[2]
            if pending_fin is not None:
                finalize(pending_fin)
                push_outproj(0)
            while op_queue:
                emit_outproj(evict_scalar=True)
    nc.compile()
    return nc


_PERM = np.concatenate(
    [np.concatenate([np.arange(0, HD, 2), np.arange(1, HD, 2)]) + h * HD
     for h in range(HPC)]
)


def _bf16(a):
    import ml_dtypes

    return np.ascontiguousarray(
        np.asarray(a, np.float32).astype(ml_dtypes.bfloat16)
    )


def _pmaj(w):
    # (D, E) weight -> partition-major (HD, NCHUNK_of_D, E)
    d, e = w.shape
    return w.reshape(d // HD, HD, e).transpose(1, 0, 2)


def prepare(x, freqs, mask, wq, wk, wv, wo):
    """Host-side sharding/prep. Returns (nc, in_maps)."""
    x = np.asarray(x, np.float32)
    freqs = np.asarray(freqs, np.float32)
    mask = np.asarray(mask, np.float32)
    wq, wk, wv, wo = (np.asarray(w, np.float32) for w in (wq, wk, wv, wo))

    statuses, maskt = _mask_structure(mask)
    nc = _build_program(statuses, maskt.shape[1])

    scale = np.float32(1.0 / np.sqrt(HD))
    cos = np.ascontiguousarray(freqs[:, :, 0].T)  # (64, T)
    sin = np.ascontiguousarray(freqs[:, :, 1].T)
    cs = np.empty((HD, 2, T), np.float32)
    cs[0:64, 0, :] = cos
    cs[64:128, 0, :] = cos
    cs[0:64, 1, :] = -sin
    cs[64:128, 1, :] = sin

    ones_col = np.ones((HD, 1), np.float32)
    # x^T in (HD, NSL, NCHUNK, SL) partition-major blocks:
    # element (p, s, k, t) = x[s*SL + t, k*HD + p]
    xts = [
        _bf16(x[b].T.reshape(NCHUNK, HD, NSL, SL).transpose(1, 2, 0, 3))
        for b in range(B)
    ]
    cs_b = _bf16(cs)
    mk_b = _bf16(maskt)
    ones_b = _bf16(ones_col)

    in_maps = []
    for core in range(8):
        b, g = core // 4, core % 4
        cols = slice(g * HPC * HD, (g + 1) * HPC * HD)
        in_maps.append({
            "xts": xts[b],
            "wqt": _bf16(_pmaj((wq.T[:, cols] * scale)[:, _PERM])),
            "wkt": _bf16(_pmaj(wk.T[:, cols][:, _PERM])),
            "wvt": _bf16(_pmaj(wv.T[:, cols])),
            "wot": _bf16(_pmaj(wo.T[cols, :])),
            "cs": cs_b,
            "maskt": mk_b,
            "ones_col": ones_b,
        })
    return nc, in_maps


def run(x, freqs, mask, wq, wk, wv, wo, **spmd_kwargs):
    nc, in_maps = prepare(x, freqs, mask, wq, wk, wv, wo)
    res = run_bass_kernel_spmd(nc, in_maps, list(range(8)), **spmd_kwargs)
    parts = [res.results[c]["out"].astype(np.float32) for c in range(8)]
    out = np.stack([
        parts[b * 4] + parts[b * 4 + 1] + parts[b * 4 + 2] + parts[b * 4 + 3]
        for b in range(B)
    ]).astype(np.float32)
    return out, res


def kernel(x, freqs, mask, wq, wk, wv, wo):
    out, _ = run(x, freqs, mask, wq, wk, wv, wo)
    return out
